# revision 1
# baseline (speedup 1.0000x reference)
"""Trainium2 Bass kernel for CausalSelfAttention (B=4, T=2048, C=768, H=6, D=128)
with RoPE + QK-RMSNorm.

Sharding: 8 cores = batch(4) x head-group(2, 3 heads each). Each core:
  - computes Q^T,K^T in (D, T) layout and V in (T, D) layout for its 3 heads
  - RoPE + RMSNorm on Q/K (partition-dim reductions via ones-matmul on PE)
  - causal attention with scores computed transposed (S^T: T_k on partitions,
    T_q on free dim) so softmax denom + AV matmuls need no transposes
  - partial c_proj over its 384 input channels
Host sums the two head-group partials per batch.
"""

import numpy as np

_B, _T, _C, _H, _D = 4, 2048, 768, 6, 128
_HPG = 3            # heads per group
_HD = _HPG * _D     # 384, per-group head dims
_NT = 4             # T tiles of 512
_TW = 512           # tile width (T_q)
_NKC = _T // 128    # 16 k-chunks of 128
_NCB = _C // 128    # 6 c_in chunks
_EPS = 1e-15

_cached = {}


def _build_nc():
    from contextlib import ExitStack
    from concourse import bacc, tile, mybir

    f32 = mybir.dt.float32
    f32r = mybir.dt.float32r
    Act = mybir.ActivationFunctionType
    Op = mybir.AluOpType

    nc = bacc.Bacc("TRN2", target_bir_lowering=False, debug=False)

    xT = nc.dram_tensor("xT", (_C, _T), f32r, kind="ExternalInput").ap()
    wq = nc.dram_tensor("wq", (_C, _HD), f32r, kind="ExternalInput").ap()
    wk = nc.dram_tensor("wk", (_C, _HD), f32r, kind="ExternalInput").ap()
    wv = nc.dram_tensor("wv", (_C, _HD), f32r, kind="ExternalInput").ap()
    wo = nc.dram_tensor("wo", (_HD, _C), f32r, kind="ExternalInput").ap()
    cc = nc.dram_tensor("cc", (128, _T), f32r, kind="ExternalInput").ap()
    ss = nc.dram_tensor("ss", (128, _T), f32r, kind="ExternalInput").ap()
    tri = nc.dram_tensor("tri", (128, 128), f32r, kind="ExternalInput").ap()
    ones = nc.dram_tensor("ones", (128, 128), f32r, kind="ExternalInput").ap()
    perm = nc.dram_tensor("perm", (128, 128), f32r, kind="ExternalInput").ap()
    out = nc.dram_tensor("out", (_T, _C), f32, kind="ExternalOutput").ap()

    with tile.TileContext(nc) as tc, ExitStack() as ctx, \
            nc.allow_low_precision(reason="f32r tiles carry full fp32 bits; PE rounds at ingest"):
        # --- pools ---
        pc = ctx.enter_context(tc.tile_pool(name="pc", bufs=1))
        pg = ctx.enter_context(tc.tile_pool(name="pg", bufs=2))         # Q tile scratch
        pa = ctx.enter_context(tc.tile_pool(name="pa", bufs=4))         # A chunks
        pz = ctx.enter_context(tc.tile_pool(name="pz", bufs=6))         # Z chunks
        psm = ctx.enter_context(tc.tile_pool(name="psm", bufs=2))       # small (1,512)/(128,512)
        pob = ctx.enter_context(tc.tile_pool(name="pob", bufs=2))       # out staging
        # psum pools (8 banks total)
        ppq = ctx.enter_context(tc.tile_pool(name="ppq", bufs=2, space="PSUM"))
        pps = ctx.enter_context(tc.tile_pool(name="pps", bufs=2, space="PSUM"))
        ppo = ctx.enter_context(tc.tile_pool(name="ppo", bufs=2, space="PSUM"))
        ppd = ctx.enter_context(tc.tile_pool(name="ppd", bufs=1, space="PSUM"))
        ppm = ctx.enter_context(tc.tile_pool(name="ppm", bufs=1, space="PSUM"))

        # --- constants / inputs resident in SBUF ---
        # load order matters: wk + xT feed the first PE work (K-projections);
        # cc/ss are not needed until rope, wq not until phase B, wo until c_proj
        t_wq, t_wk, t_wv = [], [], []
        for c in range(_NCB):
            t = pc.tile([128, _HD], f32r, tag=f"wk{c}", name=f"wk{c}",
                        padded_shape=[128, _TW])
            nc.sync.dma_start(t[:], wk[c * 128:(c + 1) * 128, :])
            t_wk.append(t)
        t_xt = []
        for c in range(_NCB):
            t = pc.tile([128, _T], f32r, tag=f"xt{c}", name=f"xt{c}")
            nc.sync.dma_start(t[:], xT[c * 128:(c + 1) * 128, :])
            t_xt.append(t)
        for c in range(_NCB):
            t = pc.tile([128, _HD], f32r, tag=f"wv{c}", name=f"wv{c}",
                        padded_shape=[128, _TW])
            nc.sync.dma_start(t[:], wv[c * 128:(c + 1) * 128, :])
            t_wv.append(t)
        t_cc = pc.tile([128, _T], f32r, tag="cc")
        t_ss = pc.tile([128, _T], f32r, tag="ss")
        nc.sync.dma_start(t_cc[:], cc[:])
        nc.sync.dma_start(t_ss[:], ss[:])
        for c in range(_NCB):
            t = pc.tile([128, _HD], f32r, tag=f"wq{c}", name=f"wq{c}")
            nc.sync.dma_start(t[:], wq[c * 128:(c + 1) * 128, :])
            t_wq.append(t)
        t_tri = pc.tile([128, 128], f32r, tag="tri")
        t_ones = pc.tile([128, 128], f32r, tag="ones")
        t_perm = pc.tile([128, 128], f32r, tag="perm")
        nc.sync.dma_start(t_tri[:], tri[:])
        nc.sync.dma_start(t_ones[:], ones[:])
        nc.sync.dma_start(t_perm[:], perm[:])
        t_ones_col = t_ones[:, 0:1]
        t_ones_row = t_ones[0:1, :]
        t_eps = pc.tile([128, 1], f32, tag="eps")
        nc.gpsimd.memset(t_eps[:], _EPS)
        t_wo = []
        for c in range(_HPG):
            t = pc.tile([128, _C], f32r, tag=f"wo{c}", name=f"wo{c}")
            nc.sync.dma_start(t[:], wo[c * 128:(c + 1) * 128, :])
            t_wo.append(t)

        # persistent K^T (post rope+norm) per head, and V blocks
        t_kn = [pc.tile([128, _T], f32r, tag=f"kn{h}", name=f"kn{h}") for h in range(_HPG)]
        t_v = [pc.tile([128, _HD], f32r, tag=f"v{tb}", name=f"v{tb}") for tb in range(_NKC)]

        def rope_part(dst_ap, col0):
            """In-place RoPE on dst_ap (128, 512)."""
            csl = slice(col0, col0 + _TW)
            p_sw = pps.tile([128, _TW], f32, tag="ps", name="p_sw")
            nc.tensor.matmul(p_sw[:], t_perm[:], dst_ap, start=True, stop=True)
            t_sw = pg.tile([128, _TW], f32r, tag="sw512", name="sw512", bufs=3)
            nc.vector.tensor_mul(dst_ap, dst_ap, t_cc[:, csl])
            nc.vector.tensor_mul(t_sw[:], p_sw[:], t_ss[:, csl])
            nc.vector.tensor_add(dst_ap, dst_ap, t_sw[:])

        def norm_pre(dst_ap, bc_pool, bc_tag, ms_on_act=True):
            """Square + partition-sum + broadcast; returns bcast psum."""
            t_sq = pg.tile([128, _TW], f32r, tag="sq512", name="sq512", bufs=3)
            nc.vector.tensor_mul(t_sq[:], dst_ap, dst_ap)
            p_ms = ppm.tile([1, _TW], f32, tag="pms", name="p_ms")
            nc.tensor.matmul(p_ms[:], t_ones_col, t_sq[:], start=True, stop=True)
            t_ms = psm.tile([1, _TW], f32r, tag="ms", name="t_ms", bufs=3)
            if ms_on_act:
                nc.scalar.copy(t_ms[:], p_ms[:])
            else:
                nc.vector.tensor_copy(t_ms[:], p_ms[:])
            p_bc = bc_pool.tile([128, _TW], f32, tag=bc_tag, name="p_bc")
            nc.tensor.matmul(p_bc[:], t_ones_row, t_ms[:], start=True, stop=True)
            return p_bc

        def norm_post(dst_ap, p_bc):
            """sqrt -> reciprocal -> scale, in place on dst_ap."""
            t_sd = psm.tile([128, _TW], f32r, tag="sd", name="t_sd", bufs=3)
            nc.scalar.activation(t_sd[:], p_bc[:], Act.Sqrt,
                                 bias=t_eps[:], scale=1.0 / 128.0)
            nc.vector.reciprocal(t_sd[:], t_sd[:])
            nc.vector.tensor_mul(dst_ap, dst_ap, t_sd[:])

        # one bcast-psum route per head so three chains can be in flight
        _bc_routes = [(pps, "ps"), (ppq, "pq"), (ppd, "pd")]

        def rope_norm(dst_ap, tw, col0):
            rope_part(dst_ap, col0)
            norm_post(dst_ap, norm_pre(dst_ap, pps, "ps"))

        # ---------------- Phase A: K^T (rope+norm) and V ----------------
        for i in range(_NT):
            isl = slice(i * _TW, (i + 1) * _TW)
            for h in range(_HPG):
                hsl = slice(h * 128, (h + 1) * 128)
                p_k = pps.tile([128, _TW], f32, tag="ps")
                for c in range(_NCB):
                    nc.tensor.matmul(p_k[:], t_wk[c][:, hsl], t_xt[c][:, isl],
                                     start=(c == 0), stop=(c == _NCB - 1))
                nc.scalar.copy(t_kn[h][:, isl], p_k[:])
        # V-projs emitted here: independent PE work that fills the gaps in
        # the serial rope+norm chains below
        for tb in range(_NKC):
            bsl = slice(tb * 128, (tb + 1) * 128)
            p_v = ppo.tile([128, _HD], f32, tag="po")
            for c in range(_NCB):
                nc.tensor.matmul(p_v[:], t_xt[c][:, bsl], t_wv[c][:],
                                 start=(c == 0), stop=(c == _NCB - 1))
            nc.scalar.copy(t_v[tb][:], p_v[:])
        # stage-batched across heads: three chains in flight, each using its
        # own bcast-psum pool (ppq/ppd are otherwise idle in phase A)
        for i in range(_NT):
            isl = slice(i * _TW, (i + 1) * _TW)
            for h in range(_HPG):
                rope_part(t_kn[h][:, isl], i * _TW)
            bcs = []
            for h in range(_HPG):
                pool, tag = _bc_routes[h]
                bcs.append(norm_pre(t_kn[h][:, isl], pool, tag))
            for h in range(_HPG):
                norm_post(t_kn[h][:, isl], bcs[h])

        # ---------------- Phase B: per T_q tile ----------------
        a_ctr = [0]

        def q_chain(qt, h):
            qsl = slice(qt * _TW, (qt + 1) * _TW)
            hsl = slice(h * 128, (h + 1) * 128)
            p_q = ppq.tile([128, _TW], f32, tag="pq", name="p_q")
            for c in range(_NCB):
                nc.tensor.matmul(p_q[:], t_wq[c][:, hsl], t_xt[c][:, qsl],
                                 start=(c == 0), stop=(c == _NCB - 1))
            t_g = pg.tile([128, _TW], f32r, tag="g", name="g", bufs=7)
            nc.vector.tensor_copy(t_g[:], p_q[:])
            rope_part(t_g[:], qt * _TW)
            pool, tag = _bc_routes[h] if h < 2 else (pps, "ps")
            norm_post(t_g[:], norm_pre(t_g[:], pool, tag, ms_on_act=False))
            return t_g

        def attention(qt, h, t_g):
            """Causal attention for one (T_q tile, head). The den/AV matmuls
            are emitted LOOKAHEAD chunks behind the S/exp pair: the PE stream
            is in-order, so den(kc) stalls on exp(kc) unless later S-matmuls
            are issued first."""
            hsl = slice(h * 128, (h + 1) * 128)
            nchunk = 4 * qt + 4
            LOOKAHEAD = 3
            p_den = ppd.tile([1, _TW], f32, tag="pd", name="p_den")
            p_o = ppo.tile([128, _TW], f32, tag="po", name="p_o")
            a_tiles = {}

            def emit_s(kc):
                roff = 0 if kc < 4 * qt else (kc - 4 * qt) * 128
                nsl = slice(roff, _TW)
                ksl = slice(kc * 128, (kc + 1) * 128)
                p_s = pps.tile([128, _TW], f32, tag="ps", name="p_s")
                nc.tensor.matmul(p_s[:, nsl], t_kn[h][:, ksl], t_g[:, nsl],
                                 start=True, stop=True)
                t_a = pc.tile([128, _TW], f32r, tag=f"wk{a_ctr[0] % _NCB}",
                              name=f"a{a_ctr[0] % _NCB}")
                a_ctr[0] += 1
                nc.scalar.activation(t_a[:, nsl], p_s[:, nsl], Act.Exp,
                                     scale=1.0 / float(np.sqrt(_D)))
                if kc >= 4 * qt:  # diagonal chunk: triangular mask
                    dsl = slice(roff, roff + 128)
                    nc.vector.tensor_mul(t_a[:, dsl], t_a[:, dsl], t_tri[:])
                a_tiles[kc] = t_a

            def emit_acc(kc):
                roff = 0 if kc < 4 * qt else (kc - 4 * qt) * 128
                nsl = slice(roff, _TW)
                t_a = a_tiles.pop(kc)
                nc.tensor.matmul(p_den[:, nsl], t_ones_col, t_a[:, nsl],
                                 start=(kc == 0), stop=(kc == nchunk - 1))
                nc.tensor.matmul(p_o[:, nsl], t_v[kc][:, hsl], t_a[:, nsl],
                                 start=(kc == 0), stop=(kc == nchunk - 1))

            for kc in range(nchunk + LOOKAHEAD):
                if kc < nchunk:
                    emit_s(kc)
                if kc >= LOOKAHEAD:
                    emit_acc(kc - LOOKAHEAD)
            # normalize: Z = O_unnorm * (1/den) broadcast
            t_den = psm.tile([1, _TW], f32r, tag="ms", name="t_den", bufs=3)
            nc.scalar.copy(t_den[:], p_den[:])
            p_db = pps.tile([128, _TW], f32, tag="ps", name="p_db")
            nc.tensor.matmul(p_db[:], t_ones_row, t_den[:], start=True, stop=True)
            t_rc2 = psm.tile([128, _TW], f32r, tag="sd", name="t_rc2", bufs=3)
            nc.vector.reciprocal(t_rc2[:], p_db[:])
            zi = h + _HPG * (qt % 2)
            t_z = pc.tile([128, _TW], f32r, tag=f"wv{zi}", name=f"z{zi}")
            nc.vector.tensor_mul(t_z[:], p_o[:], t_rc2[:])
            return t_z

        for qt in range(_NT):
            gs = [q_chain(qt, h) for h in range(_HPG)]
            z_chunks = [attention(qt, h, gs[h]) for h in range(_HPG)]
            # c_proj for this tile: partial out rows [qt*512, qt*512+512)
            for tb in range(4):
                bsl = slice(tb * 128, (tb + 1) * 128)
                t_ob = pob.tile([128, _C], f32, tag="ob")
                for nh in range(2):
                    osl = slice(nh * 384, (nh + 1) * 384)
                    p_c = ppq.tile([128, 384], f32, tag="pq")
                    for c in range(_HPG):
                        nc.tensor.matmul(p_c[:], z_chunks[c][:, bsl],
                                         t_wo[c][:, osl],
                                         start=(c == 0), stop=(c == _HPG - 1))
                    nc.vector.tensor_copy(t_ob[:, osl], p_c[:])
                nc.sync.dma_start(
                    out[qt * _TW + tb * 128: qt * _TW + (tb + 1) * 128, :],
                    t_ob[:])

    nc.compile()
    return nc


def _get_nc():
    if "nc" not in _cached:
        _cached["nc"] = _build_nc()
    return _cached["nc"]


def make_in_maps(x, cos, sin, Wq, Wk, Wv, Wo):
    cosT = np.ascontiguousarray(cos.reshape(_T, _D // 2).T)  # (64, T)
    sinT = np.ascontiguousarray(sin.reshape(_T, _D // 2).T)
    cc = np.concatenate([cosT, cosT], axis=0)                # (128, T)
    ss = np.concatenate([sinT, -sinT], axis=0)
    tri = (np.arange(128)[None, :] >= np.arange(128)[:, None]).astype(np.float32)
    ones128 = np.ones((128, 128), dtype=np.float32)
    permm = np.zeros((128, 128), dtype=np.float32)           # half-swap permutation
    for d in range(64):
        permm[64 + d, d] = 1.0
        permm[d, 64 + d] = 1.0
    in_maps = []
    for core in range(8):
        b, g = divmod(core, 2)
        gsl = slice(g * _HD, (g + 1) * _HD)
        in_maps.append({
            "xT": np.ascontiguousarray(x[b].T),
            "wq": np.ascontiguousarray(Wq[gsl, :].T),
            "wk": np.ascontiguousarray(Wk[gsl, :].T),
            "wv": np.ascontiguousarray(Wv[gsl, :].T),
            "wo": np.ascontiguousarray(Wo[:, gsl].T),
            "cc": cc, "ss": ss, "tri": tri, "ones": ones128, "perm": permm,
        })
    return in_maps


def kernel(x, cos, sin, Wq, Wk, Wv, Wo):
    from concourse.bass_utils import run_bass_kernel_spmd

    x = np.asarray(x, dtype=np.float32)
    cos = np.asarray(cos, dtype=np.float32)
    sin = np.asarray(sin, dtype=np.float32)
    Wq = np.asarray(Wq, dtype=np.float32)
    Wk = np.asarray(Wk, dtype=np.float32)
    Wv = np.asarray(Wv, dtype=np.float32)
    Wo = np.asarray(Wo, dtype=np.float32)

    nc = _get_nc()
    in_maps = make_in_maps(x, cos, sin, Wq, Wk, Wv, Wo)
    res = run_bass_kernel_spmd(nc, in_maps, core_ids=list(range(8)))
    outs = [r_["out"] for r_ in res.results]
    return np.stack([outs[2 * b] + outs[2 * b + 1] for b in range(_B)], axis=0)



# revision 3
# speedup vs baseline: 1.8741x; 1.8741x over previous
"""Trainium2 Bass kernel for CausalSelfAttention (B=4, T=2048, C=768, H=6, D=128)
with RoPE + QK-RMSNorm.  v3: one act-table set, soft-pipelined phases.

Sharding: 8 cores = batch(4) x head-group(2, 3 heads each).

Key points:
  - Q/K/A/V tiles bf16 (DVE 2x, same PE rate); projections/c_proj f32r.
  - RoPE half-swap via partition-offset DVE muls (no PE perm matmul).
  - K-side RMSNorm never scales K: rk = 1/sqrt(ms_k) columns (with 1/sqrt(D)
    folded in) feed exp's per-partition scale.  ms_k columns computed directly
    via matmul(lhsT=sq_chunk, rhs=ones_col) at psum partitions.
  - Q-side RMSNorm via gpsimd partition_all_reduce.
  - rsqrt = exp(-0.5*ln(x)) on Act; every activation func lives in act-table
    set 'natural_log_exp_and_others', pinned via a filtered table view, so
    the scheduler can interleave norm chains with attention exps freely with
    zero table reloads.
  - Softmax den rows for 3 heads pack one PSUM bank at partitions 0/32/64.
"""

import numpy as np

_B, _T, _C, _H, _D = 4, 2048, 768, 6, 128
_HPG = 3            # heads per group (per core)
_HD = _HPG * _D     # 384
_NT = 4             # T tiles of 512
_TW = 512
_NKC = _T // 128    # 16 k-chunks
_NCB = _C // 128    # 6 contraction chunks
_EPS = 1e-15

_cached = {}


def _patch_act_tables():
    """Pin every activation func we use to the 'natural_log_exp_and_others'
    table: present a filtered view to Bacc's table-load pass in which a func
    appears in a non-6 set only if set 6 cannot serve it.  Runtime behavior is
    unchanged (the real set 6 does contain ln/exp/copy/square/identity); this
    only steers load placement so ln/exp alternation never reloads."""
    import concourse.bacc as bacc_mod
    import concourse.hw_specs as hw_mod
    if getattr(bacc_mod, "_act_tables_patched", False):
        return
    orig = hw_mod.get_activation_tables

    def patched(arch):
        tables = orig(arch)
        items = list(tables.items())
        target = None
        for name, funcs in items:
            if name == "natural_log_exp_and_others":
                target = funcs
        if target is None:
            return tables
        out = {}
        for name, funcs in items:
            if name == "natural_log_exp_and_others":
                out[name] = funcs
            else:
                out[name] = {f for f in funcs if f not in target}
        return out

    bacc_mod.get_activation_tables = patched
    bacc_mod._act_tables_patched = True


def _build_nc():
    from contextlib import ExitStack
    from concourse import bacc, tile, mybir, bass_isa

    _patch_act_tables()

    f32 = mybir.dt.float32
    f32r = mybir.dt.float32r
    bf16 = mybir.dt.bfloat16
    Act = mybir.ActivationFunctionType

    nc = bacc.Bacc("TRN2", target_bir_lowering=False, debug=False)

    xT = nc.dram_tensor("xT", (_C, _T), bf16, kind="ExternalInput").ap()
    wq = nc.dram_tensor("wq", (_C, _HD), bf16, kind="ExternalInput").ap()
    wk = nc.dram_tensor("wk", (_C, _HD), bf16, kind="ExternalInput").ap()
    wv = nc.dram_tensor("wv", (_C, _HD), bf16, kind="ExternalInput").ap()
    wo = nc.dram_tensor("wo", (_HD, _C), bf16, kind="ExternalInput").ap()
    cc = nc.dram_tensor("cc", (128, _T), bf16, kind="ExternalInput").ap()
    ssr = nc.dram_tensor("ssr", (128, _T), bf16, kind="ExternalInput").ap()
    tri = nc.dram_tensor("tri", (128, 128), bf16, kind="ExternalInput").ap()
    onesb = nc.dram_tensor("onesb", (128, 128), bf16, kind="ExternalInput").ap()
    out = nc.dram_tensor("out", (_T, _C), f32, kind="ExternalOutput").ap()

    with tile.TileContext(nc) as tc, ExitStack() as ctx, \
            nc.allow_low_precision(reason="bf16 attention pipeline, f32 accum"):
        # ---------------- pools ----------------
        pc = ctx.enter_context(tc.tile_pool(name="pc", bufs=1))    # persistent
        pg = ctx.enter_context(tc.tile_pool(name="pg", bufs=2))    # scratch
        pa = ctx.enter_context(tc.tile_pool(name="pa", bufs=2))    # A tiles
        # PSUM: 2 + 2 + 3 + 1 = 8 banks
        ppj = ctx.enter_context(tc.tile_pool(name="ppj", bufs=2, space="PSUM"))
        pps = ctx.enter_context(tc.tile_pool(name="pps", bufs=2, space="PSUM"))
        ppo = ctx.enter_context(tc.tile_pool(name="ppo", bufs=3, space="PSUM"))
        ppd = ctx.enter_context(tc.tile_pool(name="ppd", bufs=1, space="PSUM"))

        # ---------------- persistent SBUF tiles + loads ----------------
        # single-DMA weight loads (3D APs) so the SP ring's fixed per-DMA
        # cost never gates the first projection groups.
        t_wkall = pc.tile([128, _NCB * _HD], bf16, tag="wkall")
        t_wqall = pc.tile([128, _NCB * _HD], bf16, tag="wqall")
        t_wvall = pc.tile([128, _NCB * _HD], bf16, tag="wvall")
        t_woall = pc.tile([128, _HPG * _C], bf16, tag="woall")
        t_wk = [t_wkall[:, c * _HD:(c + 1) * _HD] for c in range(_NCB)]
        t_wq = [t_wqall[:, c * _HD:(c + 1) * _HD] for c in range(_NCB)]
        t_wv = [t_wvall[:, c * _HD:(c + 1) * _HD] for c in range(_NCB)]
        t_wo = [t_woall[:, c * _C:(c + 1) * _C] for c in range(_HPG)]

        def _wload(tile_, dram, nchunks, width):
            nc.sync.dma_start(
                tile_[:].rearrange("p (c f) -> p c f", c=nchunks),
                dram.rearrange("(c p) f -> p c f", p=128))

        t_xt = [pc.tile([128, _T], bf16, tag=f"xt{c}", name=f"xt{c}")
                for c in range(_NCB)]
        t_cc = pc.tile([128, _T], bf16, tag="cc")
        t_ssr = pc.tile([128, _T], bf16, tag="ssr")
        t_tri = pc.tile([128, 128], bf16, tag="tri")
        t_ones = pc.tile([128, 128], bf16, tag="ones")

        def _xt_load(i):
            isl = slice(i * _TW, (i + 1) * _TW)
            for c in range(_NCB):
                nc.sync.dma_start(t_xt[c][:, isl], xT[c * 128:(c + 1) * 128, isl])

        _wload(t_wkall, wk, _NCB, _HD)
        _xt_load(0)
        _wload(t_wqall, wq, _NCB, _HD)
        nc.sync.dma_start(t_cc[:], cc[:])
        nc.sync.dma_start(t_ssr[:], ssr[:])
        nc.sync.dma_start(t_ones[:], onesb[:])
        _xt_load(1)
        _wload(t_wvall, wv, _NCB, _HD)
        _xt_load(2)
        nc.sync.dma_start(t_tri[:], tri[:])
        _wload(t_woall, wo, _HPG, _C)
        _xt_load(3)

        t_onescol = t_ones[:, 0:1]
        t_eps = pc.tile([128, 1], f32, tag="eps")
        nc.gpsimd.memset(t_eps[:], _EPS)

        # persistent per-head tensors
        t_kn = [pc.tile([128, _T], bf16, tag=f"kn{h}", name=f"kn{h}")
                for h in range(_HPG)]
        t_q = [pc.tile([128, _T], bf16, tag=f"q{h}", name=f"q{h}")
               for h in range(_HPG)]
        t_v = [pc.tile([128, _HD], bf16, tag=f"v{tb}", name=f"v{tb}")
               for tb in range(_NKC)]
        t_rk = [pc.tile([128, _NKC], f32, tag=f"rk{h}", name=f"rk{h}")
                for h in range(_HPG)]

        # ================ projections + rope + norms ================
        def proj_group(ws, h, isl, tag, halves=False):
            hsl = slice(h * 128, (h + 1) * 128)
            p = ppj.tile([128, _TW], f32, tag="pj", name=f"p{tag}")
            if halves:
                for colsl in (slice(0, 256), slice(256, 512)):
                    for c in range(_NCB):
                        nc.tensor.matmul(p[:, colsl], ws[c][:, hsl],
                                         t_xt[c][:, isl][:, colsl],
                                         start=(c == 0), stop=(c == _NCB - 1))
            else:
                for c in range(_NCB):
                    nc.tensor.matmul(p[:], ws[c][:, hsl], t_xt[c][:, isl],
                                     start=(c == 0), stop=(c == _NCB - 1))
            return p

        def rope_sq(dst_slice, p_raw, i, add_on_pool, evac_on_act):
            """dst = raw*cc + swap(raw)*ss (bf16); returns bf16 squares tile."""
            isl = slice(i * _TW, (i + 1) * _TW)
            t_raw = pg.tile([128, _TW], bf16, tag="raw", name="raw", bufs=4)
            if evac_on_act:
                nc.scalar.copy(t_raw[:], p_raw[:])
            else:
                nc.vector.tensor_copy(t_raw[:], p_raw[:])
            t_swm = pg.tile([128, _TW], bf16, tag="swm", name="swm", bufs=4)
            nc.vector.tensor_mul(t_swm[0:64, :], t_raw[64:128, :], t_ssr[64:128, isl])
            nc.vector.tensor_mul(t_swm[64:128, :], t_raw[0:64, :], t_ssr[0:64, isl])
            nc.vector.tensor_mul(dst_slice, t_raw[:], t_cc[:, isl])
            if add_on_pool:
                nc.gpsimd.tensor_add(dst_slice, dst_slice, t_swm[:])
            else:
                nc.vector.tensor_add(dst_slice, dst_slice, t_swm[:])
            t_sq = pg.tile([128, _TW], bf16, tag="sq", name="sq", bufs=4)
            nc.scalar.square(t_sq[:], dst_slice)
            return t_sq

        def k_chain_a(i, h):
            isl = slice(i * _TW, (i + 1) * _TW)
            p_k = proj_group(t_wk, h, isl, f"k{i}{h}")
            return rope_sq(t_kn[h][:, isl], p_k, i, add_on_pool=True, evac_on_act=True)

        def k_chain_b(i, h, t_sq):
            # ms_k columns (128,4) then rk = exp(-0.5*ln(ms_k))
            p_cols = pps.tile([128, 4], f32, tag="ps", name=f"cols{i}{h}",
                              padded_shape=[128, _TW])
            for j in range(4):
                nc.tensor.matmul(p_cols[:, j:j + 1], t_sq[:, j * 128:(j + 1) * 128],
                                 t_onescol, start=True, stop=True)
            t_lnk = pg.tile([128, 4], f32, tag="lnk", name="lnk", bufs=4)
            nc.scalar.activation(t_lnk[:], p_cols[:], Act.Ln,
                                 bias=t_eps[:], scale=1.0)
            nc.scalar.activation(t_rk[h][:, 4 * i:4 * i + 4], t_lnk[:], Act.Exp,
                                 bias=0.0, scale=-0.5)

        def q_chain(i, h):
            isl = slice(i * _TW, (i + 1) * _TW)
            p_q = proj_group(t_wq, h, isl, f"q{i}{h}")
            t_sq = rope_sq(t_q[h][:, isl], p_q, i, add_on_pool=False, evac_on_act=False)
            t_ms = pg.tile([128, _TW], bf16, tag="ms", name="ms", bufs=2)
            nc.gpsimd.partition_all_reduce(t_ms[:], t_sq[:], channels=128,
                                           reduce_op=bass_isa.ReduceOp.add)
            t_ln = pg.tile([128, _TW], bf16, tag="qln", name="qln", bufs=2)
            nc.scalar.activation(t_ln[:], t_ms[:], Act.Ln,
                                 bias=t_eps[:], scale=1.0 / 128.0)
            t_r = pg.tile([128, _TW], bf16, tag="qr", name="qr", bufs=2)
            nc.scalar.activation(t_r[:], t_ln[:], Act.Exp, bias=0.0, scale=-0.5)
            nc.vector.tensor_mul(t_q[h][:, isl], t_q[h][:, isl], t_r[:])

        def v_group(tb):
            bsl = slice(tb * 128, (tb + 1) * 128)
            p_v = ppj.tile([128, _HD], f32, tag="pj", name=f"pv{tb}",
                           padded_shape=[128, _TW])
            for c in range(_NCB):
                nc.tensor.matmul(p_v[:], t_xt[c][:, bsl], t_wv[c][:],
                                 start=(c == 0), stop=(c == _NCB - 1))
            if tb % 2 == 0:
                nc.vector.tensor_copy(t_v[tb][:], p_v[:])
            else:
                nc.scalar.copy(t_v[tb][:], p_v[:])

        # ================ attention + c_proj ================
        def attention(qt):
            nchunk = 4 * qt + 4
            LOOKAHEAD = 3
            qsl = slice(qt * _TW, (qt + 1) * _TW)
            p_den = ppd.tile([128, _TW], f32, tag="pd", name=f"pd{qt}")
            p_os = [ppo.tile([128, _TW], f32, tag="po", name=f"po{qt}{h}")
                    for h in range(_HPG)]
            a_tiles = {}

            def emit_s(kc, h):
                roff = 0 if kc < 4 * qt else (kc - 4 * qt) * 128
                nsl = slice(roff, _TW)
                ksl = slice(kc * 128, (kc + 1) * 128)
                p_s = pps.tile([128, _TW], f32, tag="ps", name=f"s{kc}{h}")
                nc.tensor.matmul(p_s[:, nsl], t_kn[h][:, ksl],
                                 t_q[h][:, qsl][:, nsl], start=True, stop=True)
                t_a = pa.tile([128, _TW], bf16, tag="a", name="a", bufs=16)
                nc.scalar.activation(t_a[:, nsl], p_s[:, nsl], Act.Exp,
                                     bias=0.0, scale=t_rk[h][:, kc:kc + 1])
                if kc >= 4 * qt:
                    dsl = slice(roff, roff + 128)
                    nc.vector.tensor_mul(t_a[:, dsl], t_a[:, dsl], t_tri[:])
                a_tiles[(kc, h)] = t_a

            def emit_acc(kc, h):
                roff = 0 if kc < 4 * qt else (kc - 4 * qt) * 128
                nsl = slice(roff, _TW)
                hsl = slice(h * 128, (h + 1) * 128)
                t_a = a_tiles.pop((kc, h))
                nc.tensor.matmul(p_den[32 * h:32 * h + 1, nsl], t_onescol,
                                 t_a[:, nsl],
                                 start=(kc == 0), stop=(kc == nchunk - 1))
                nc.tensor.matmul(p_os[h][:, nsl], t_v[kc][:, hsl], t_a[:, nsl],
                                 start=(kc == 0), stop=(kc == nchunk - 1))

            t_zs = [None] * _HPG

            def emit_z(h):
                t_rd = pg.tile([1, _TW], f32, tag="rd", name="rd", bufs=3)
                nc.vector.reciprocal(t_rd[:], p_den[32 * h:32 * h + 1, :])
                t_rdb = pg.tile([128, _TW], f32, tag="rdb", name="rdb", bufs=3)
                nc.gpsimd.partition_broadcast(t_rdb[:], t_rd[:])
                t_z = pg.tile([128, _TW], bf16, tag=f"z{h}", name=f"z{h}", bufs=2)
                nc.vector.tensor_mul(t_z[:], p_os[h][:], t_rdb[:])
                t_zs[h] = t_z

            # heads skewed by SKEW chunks so they finish staggered: each
            # head's z-chain (recip->bcast->mul, ~2.5us) runs while later
            # heads still stream chunks, freeing p_o banks incrementally.
            SKEW = 2
            for v in range(nchunk + 2 * SKEW + LOOKAHEAD + 1):
                for h in range(_HPG):
                    kc_s = v - SKEW * h
                    if 0 <= kc_s < nchunk:
                        emit_s(kc_s, h)
                    kc_a = v - SKEW * h - LOOKAHEAD
                    if 0 <= kc_a < nchunk:
                        emit_acc(kc_a, h)
                        if kc_a == nchunk - 1:
                            emit_z(h)
            return t_zs

        def c_proj(qt, t_zs):
            for tb in range(4):
                bsl = slice(tb * 128, (tb + 1) * 128)
                r0 = qt * _TW + tb * 128
                t_ob = pg.tile([128, _C], f32, tag="ob", name="ob", bufs=3)
                for nh in range(2):
                    osl = slice(nh * 384, (nh + 1) * 384)
                    p_c = ppj.tile([128, 384], f32, tag="pj", name=f"pc{qt}{tb}{nh}",
                                   padded_shape=[128, _TW])
                    for c in range(_HPG):
                        nc.tensor.matmul(p_c[:], t_zs[c][:, bsl], t_wo[c][:, osl],
                                         start=(c == 0), stop=(c == _HPG - 1))
                    if nh == 0:
                        nc.vector.tensor_copy(t_ob[:, osl], p_c[:])
                    else:
                        nc.scalar.copy(t_ob[:, osl], p_c[:])
                eng = nc.sync if tb % 2 == 0 else nc.scalar
                eng.dma_start(out[r0:r0 + 128, :], t_ob[:])

        # ---------------- emission ----------------
        for i in range(_NT):
            k_sqs = [k_chain_a(i, h) for h in range(_HPG)]
            for h in range(_HPG):
                q_chain(i, h)
            for tb in range(4 * i, 4 * i + 4):
                v_group(tb)
            for h in range(_HPG):
                k_chain_b(i, h, k_sqs[h])

        for qt in range(_NT):
            t_zs = attention(qt)
            c_proj(qt, t_zs)

    nc.compile()
    return nc


def _get_nc():
    if "nc" not in _cached:
        _cached["nc"] = _build_nc()
    return _cached["nc"]


def make_in_maps(x, cos, sin, Wq, Wk, Wv, Wo):
    import ml_dtypes
    bf = ml_dtypes.bfloat16
    cosT = np.ascontiguousarray(cos.reshape(_T, _D // 2).T)  # (64, T)
    sinT = np.ascontiguousarray(sin.reshape(_T, _D // 2).T)
    ccm = np.concatenate([cosT, cosT], axis=0).astype(bf)     # (128, T)
    ssm = np.concatenate([sinT, -sinT], axis=0).astype(bf)
    ssrm = np.concatenate([-sinT, sinT], axis=0).astype(bf)
    trim = (np.arange(128)[None, :] >= np.arange(128)[:, None]).astype(bf)
    ones128 = np.ones((128, 128), dtype=bf)
    in_maps = []
    for core in range(8):
        b, g = divmod(core, 2)
        gsl = slice(g * _HD, (g + 1) * _HD)
        in_maps.append({
            "xT": np.ascontiguousarray(x[b].T).astype(bf),
            "wq": np.ascontiguousarray(Wq[gsl, :].T).astype(bf),
            "wk": np.ascontiguousarray(Wk[gsl, :].T).astype(bf),
            "wv": np.ascontiguousarray(Wv[gsl, :].T).astype(bf),
            "wo": np.ascontiguousarray(Wo[:, gsl].T).astype(bf),
            "cc": ccm, "ssr": ssrm, "tri": trim, "onesb": ones128,
        })
    return in_maps


def kernel(x, cos, sin, Wq, Wk, Wv, Wo):
    from concourse.bass_utils import run_bass_kernel_spmd

    x = np.asarray(x, dtype=np.float32)
    cos = np.asarray(cos, dtype=np.float32)
    sin = np.asarray(sin, dtype=np.float32)
    Wq = np.asarray(Wq, dtype=np.float32)
    Wk = np.asarray(Wk, dtype=np.float32)
    Wv = np.asarray(Wv, dtype=np.float32)
    Wo = np.asarray(Wo, dtype=np.float32)

    nc = _get_nc()
    in_maps = make_in_maps(x, cos, sin, Wq, Wk, Wv, Wo)
    res = run_bass_kernel_spmd(nc, in_maps, core_ids=list(range(8)))
    outs = [r_["out"] for r_ in res.results]
    return np.stack([outs[2 * b] + outs[2 * b + 1] for b in range(_B)], axis=0)


# revision 4
# speedup vs baseline: 1.9467x; 1.0387x over previous
"""Trainium2 Bass kernel for CausalSelfAttention (B=4, T=2048, C=768, H=6, D=128)
with RoPE + QK-RMSNorm.  v3: one act-table set, soft-pipelined phases.

Sharding: 8 cores = batch(4) x head-group(2, 3 heads each).

Key points:
  - Q/K/A/V tiles bf16 (DVE 2x, same PE rate); projections/c_proj f32r.
  - RoPE half-swap via partition-offset DVE muls (no PE perm matmul).
  - K-side RMSNorm never scales K: rk = 1/sqrt(ms_k) columns (with 1/sqrt(D)
    folded in) feed exp's per-partition scale.  ms_k columns computed directly
    via matmul(lhsT=sq_chunk, rhs=ones_col) at psum partitions.
  - Q-side RMSNorm via gpsimd partition_all_reduce.
  - rsqrt = exp(-0.5*ln(x)) on Act; every activation func lives in act-table
    set 'natural_log_exp_and_others', pinned via a filtered table view, so
    the scheduler can interleave norm chains with attention exps freely with
    zero table reloads.
  - Softmax den rows for 3 heads pack one PSUM bank at partitions 0/32/64.
"""

import numpy as np

_B, _T, _C, _H, _D = 4, 2048, 768, 6, 128
_HPG = 3            # heads per group (per core)
_HD = _HPG * _D     # 384
_NT = 4             # T tiles of 512
_TW = 512
_NKC = _T // 128    # 16 k-chunks
_NCB = _C // 128    # 6 contraction chunks
_EPS = 1e-15

_cached = {}


def _patch_act_tables():
    """Pin every activation func we use to the 'natural_log_exp_and_others'
    table: present a filtered view to Bacc's table-load pass in which a func
    appears in a non-6 set only if set 6 cannot serve it.  Runtime behavior is
    unchanged (the real set 6 does contain ln/exp/copy/square/identity); this
    only steers load placement so ln/exp alternation never reloads."""
    import concourse.bacc as bacc_mod
    import concourse.hw_specs as hw_mod
    if getattr(bacc_mod, "_act_tables_patched", False):
        return
    orig = hw_mod.get_activation_tables

    def patched(arch):
        tables = orig(arch)
        items = list(tables.items())
        target = None
        for name, funcs in items:
            if name == "natural_log_exp_and_others":
                target = funcs
        if target is None:
            return tables
        out = {}
        for name, funcs in items:
            if name == "natural_log_exp_and_others":
                out[name] = funcs
            else:
                out[name] = {f for f in funcs if f not in target}
        return out

    bacc_mod.get_activation_tables = patched
    bacc_mod._act_tables_patched = True


def _build_nc():
    from contextlib import ExitStack
    from concourse import bacc, tile, mybir, bass_isa

    _patch_act_tables()

    f32 = mybir.dt.float32
    f32r = mybir.dt.float32r
    bf16 = mybir.dt.bfloat16
    Act = mybir.ActivationFunctionType

    nc = bacc.Bacc("TRN2", target_bir_lowering=False, debug=False)

    xT = nc.dram_tensor("xT", (_C, _T), bf16, kind="ExternalInput").ap()
    wq = nc.dram_tensor("wq", (_C, _HD), bf16, kind="ExternalInput").ap()
    wk = nc.dram_tensor("wk", (_C, _HD), bf16, kind="ExternalInput").ap()
    wv = nc.dram_tensor("wv", (_C, _HD), bf16, kind="ExternalInput").ap()
    wo = nc.dram_tensor("wo", (_HD, _C), bf16, kind="ExternalInput").ap()
    cc = nc.dram_tensor("cc", (128, _T), bf16, kind="ExternalInput").ap()
    ssr = nc.dram_tensor("ssr", (128, _T), bf16, kind="ExternalInput").ap()
    tri = nc.dram_tensor("tri", (128, 128), bf16, kind="ExternalInput").ap()
    onesb = nc.dram_tensor("onesb", (128, 128), bf16, kind="ExternalInput").ap()
    out = nc.dram_tensor("out", (_T, _C), bf16, kind="ExternalOutput").ap()

    with tile.TileContext(nc) as tc, ExitStack() as ctx, \
            nc.allow_low_precision(reason="bf16 attention pipeline, f32 accum"):
        # ---------------- pools ----------------
        pc = ctx.enter_context(tc.tile_pool(name="pc", bufs=1))    # persistent
        pg = ctx.enter_context(tc.tile_pool(name="pg", bufs=2))    # scratch
        pa = ctx.enter_context(tc.tile_pool(name="pa", bufs=2))    # A tiles
        # PSUM: 2 + 2 + 3 + 1 = 8 banks
        ppj = ctx.enter_context(tc.tile_pool(name="ppj", bufs=2, space="PSUM"))
        pps = ctx.enter_context(tc.tile_pool(name="pps", bufs=2, space="PSUM"))
        ppo = ctx.enter_context(tc.tile_pool(name="ppo", bufs=3, space="PSUM"))
        ppd = ctx.enter_context(tc.tile_pool(name="ppd", bufs=1, space="PSUM"))

        # ---------------- persistent SBUF tiles + loads ----------------
        # single-DMA weight loads (3D APs) so the SP ring's fixed per-DMA
        # cost never gates the first projection groups.
        t_wkall = pc.tile([128, _NCB * _HD], bf16, tag="wkall")
        t_wqall = pc.tile([128, _NCB * _HD], bf16, tag="wqall")
        t_wvall = pc.tile([128, _NCB * _HD], bf16, tag="wvall")
        t_woall = pc.tile([128, _HPG * _C], bf16, tag="woall")
        t_wk = [t_wkall[:, c * _HD:(c + 1) * _HD] for c in range(_NCB)]
        t_wq = [t_wqall[:, c * _HD:(c + 1) * _HD] for c in range(_NCB)]
        t_wv = [t_wvall[:, c * _HD:(c + 1) * _HD] for c in range(_NCB)]
        t_wo = [t_woall[:, c * _C:(c + 1) * _C] for c in range(_HPG)]

        def _wload(tile_, dram, nchunks, width):
            nc.sync.dma_start(
                tile_[:].rearrange("p (c f) -> p c f", c=nchunks),
                dram.rearrange("(c p) f -> p c f", p=128))

        t_xt = [pc.tile([128, _T], bf16, tag=f"xt{c}", name=f"xt{c}")
                for c in range(_NCB)]
        t_cc = pc.tile([128, _T], bf16, tag="cc")
        t_ssr = pc.tile([128, _T], bf16, tag="ssr")
        t_tri = pc.tile([128, 128], bf16, tag="tri")
        t_ones = pc.tile([128, 128], bf16, tag="ones")

        def _xt_load(i):
            isl = slice(i * _TW, (i + 1) * _TW)
            for c in range(_NCB):
                nc.sync.dma_start(t_xt[c][:, isl], xT[c * 128:(c + 1) * 128, isl])

        _wload(t_wkall, wk, _NCB, _HD)
        _xt_load(0)
        _wload(t_wqall, wq, _NCB, _HD)
        nc.sync.dma_start(t_cc[:], cc[:])
        nc.sync.dma_start(t_ssr[:], ssr[:])
        nc.sync.dma_start(t_ones[:], onesb[:])
        _xt_load(1)
        _wload(t_wvall, wv, _NCB, _HD)
        _xt_load(2)
        nc.sync.dma_start(t_tri[:], tri[:])
        _wload(t_woall, wo, _HPG, _C)
        _xt_load(3)

        t_onescol = t_ones[:, 0:1]
        t_eps = pc.tile([128, 1], f32, tag="eps")
        nc.gpsimd.memset(t_eps[:], _EPS)
        # PE warmup during initial DMA wait: ramps the pstate clock and
        # fills otherwise-idle cycles; lands in the den bank (idle till qt0)
        t_warm = pc.tile([128, _TW], bf16, tag="warm")
        nc.gpsimd.memset(t_warm[:], 0.0)
        p_warm = ppd.tile([128, _TW], f32, tag="pd", name="warm")
        for _w in range(5):
            nc.tensor.matmul(p_warm[:], t_warm[:, 0:128], t_warm[:],
                             start=True, stop=True)

        # persistent per-head tensors
        t_kn = [pc.tile([128, _T], bf16, tag=f"kn{h}", name=f"kn{h}")
                for h in range(_HPG)]
        t_q = [pc.tile([128, _T], bf16, tag=f"q{h}", name=f"q{h}")
               for h in range(_HPG)]
        t_v = [pc.tile([128, _HD], bf16, tag=f"v{tb}", name=f"v{tb}")
               for tb in range(_NKC)]
        t_rk = [pc.tile([128, _NKC], f32, tag=f"rk{h}", name=f"rk{h}")
                for h in range(_HPG)]

        # ================ projections + rope + norms ================
        def proj_group(ws, h, isl, tag, halves=False):
            hsl = slice(h * 128, (h + 1) * 128)
            p = ppj.tile([128, _TW], f32, tag="pj", name=f"p{tag}")
            if halves:
                for colsl in (slice(0, 256), slice(256, 512)):
                    for c in range(_NCB):
                        nc.tensor.matmul(p[:, colsl], ws[c][:, hsl],
                                         t_xt[c][:, isl][:, colsl],
                                         start=(c == 0), stop=(c == _NCB - 1))
            else:
                for c in range(_NCB):
                    nc.tensor.matmul(p[:], ws[c][:, hsl], t_xt[c][:, isl],
                                     start=(c == 0), stop=(c == _NCB - 1))
            return p

        def rope_sq(dst_slice, p_raw, i, add_on_pool, evac_on_act):
            """dst = raw*cc + swap(raw)*ss (bf16); returns bf16 squares tile."""
            isl = slice(i * _TW, (i + 1) * _TW)
            t_raw = pg.tile([128, _TW], bf16, tag="raw", name="raw", bufs=4)
            if evac_on_act:
                nc.scalar.copy(t_raw[:], p_raw[:])
            else:
                nc.vector.tensor_copy(t_raw[:], p_raw[:])
            t_swm = pg.tile([128, _TW], bf16, tag="swm", name="swm", bufs=4)
            nc.vector.tensor_mul(t_swm[0:64, :], t_raw[64:128, :], t_ssr[64:128, isl])
            nc.vector.tensor_mul(t_swm[64:128, :], t_raw[0:64, :], t_ssr[0:64, isl])
            nc.vector.tensor_mul(dst_slice, t_raw[:], t_cc[:, isl])
            if add_on_pool:
                nc.gpsimd.tensor_add(dst_slice, dst_slice, t_swm[:])
            else:
                nc.vector.tensor_add(dst_slice, dst_slice, t_swm[:])
            t_sq = pg.tile([128, _TW], bf16, tag="sq", name="sq", bufs=4)
            if add_on_pool:
                nc.scalar.square(t_sq[:], dst_slice)
            else:
                nc.vector.tensor_mul(t_sq[:], dst_slice, dst_slice)
            return t_sq

        def k_chain_a(i, h):
            isl = slice(i * _TW, (i + 1) * _TW)
            p_k = proj_group(t_wk, h, isl, f"k{i}{h}")
            return rope_sq(t_kn[h][:, isl], p_k, i, add_on_pool=True, evac_on_act=True)

        def k_chain_b(i, h, t_sq):
            # ms_k columns (128,4) then rk = exp(-0.5*ln(ms_k))
            p_cols = pps.tile([128, 4], f32, tag="ps", name=f"cols{i}{h}",
                              padded_shape=[128, _TW])
            for j in range(4):
                nc.tensor.matmul(p_cols[:, j:j + 1], t_sq[:, j * 128:(j + 1) * 128],
                                 t_onescol, start=True, stop=True)
            t_lnk = pg.tile([128, 4], f32, tag="lnk", name="lnk", bufs=4)
            nc.scalar.activation(t_lnk[:], p_cols[:], Act.Ln,
                                 bias=t_eps[:], scale=1.0)
            nc.scalar.activation(t_rk[h][:, 4 * i:4 * i + 4], t_lnk[:], Act.Exp,
                                 bias=0.0, scale=-0.5)

        def q_chain(i, h):
            isl = slice(i * _TW, (i + 1) * _TW)
            p_q = proj_group(t_wq, h, isl, f"q{i}{h}")
            t_sq = rope_sq(t_q[h][:, isl], p_q, i, add_on_pool=False, evac_on_act=False)
            t_ms = pg.tile([128, _TW], bf16, tag="ms", name="ms", bufs=2)
            nc.gpsimd.partition_all_reduce(t_ms[:], t_sq[:], channels=128,
                                           reduce_op=bass_isa.ReduceOp.add)
            t_ln = pg.tile([128, _TW], bf16, tag="qln", name="qln", bufs=2)
            nc.scalar.activation(t_ln[:], t_ms[:], Act.Ln,
                                 bias=t_eps[:], scale=1.0 / 128.0)
            t_r = pg.tile([128, _TW], bf16, tag="qr", name="qr", bufs=2)
            nc.scalar.activation(t_r[:], t_ln[:], Act.Exp, bias=0.0, scale=-0.5)
            nc.vector.tensor_mul(t_q[h][:, isl], t_q[h][:, isl], t_r[:])

        def v_group(tb):
            bsl = slice(tb * 128, (tb + 1) * 128)
            p_v = ppj.tile([128, _HD], f32, tag="pj", name=f"pv{tb}",
                           padded_shape=[128, _TW])
            for c in range(_NCB):
                nc.tensor.matmul(p_v[:], t_xt[c][:, bsl], t_wv[c][:],
                                 start=(c == 0), stop=(c == _NCB - 1))
            if tb % 2 == 0:
                nc.vector.tensor_copy(t_v[tb][:], p_v[:])
            else:
                nc.scalar.copy(t_v[tb][:], p_v[:])

        # ================ attention + c_proj ================
        def attention(qt):
            nchunk = 4 * qt + 4
            LOOKAHEAD = 4
            qsl = slice(qt * _TW, (qt + 1) * _TW)
            p_den = ppd.tile([128, _TW], f32, tag="pd", name=f"pd{qt}")
            p_os = [ppo.tile([128, _TW], f32, tag="po", name=f"po{qt}{h}")
                    for h in range(_HPG)]
            a_tiles = {}

            def emit_s(kc, h):
                roff = 0 if kc < 4 * qt else (kc - 4 * qt) * 128
                nsl = slice(roff, _TW)
                ksl = slice(kc * 128, (kc + 1) * 128)
                p_s = pps.tile([128, _TW], f32, tag="ps", name=f"s{kc}{h}")
                nc.tensor.matmul(p_s[:, nsl], t_kn[h][:, ksl],
                                 t_q[h][:, qsl][:, nsl], start=True, stop=True)
                t_a = pa.tile([128, _TW], bf16, tag="a", name="a", bufs=16)
                nc.scalar.activation(t_a[:, nsl], p_s[:, nsl], Act.Exp,
                                     bias=0.0, scale=t_rk[h][:, kc:kc + 1])
                if kc >= 4 * qt:
                    dsl = slice(roff, roff + 128)
                    nc.vector.tensor_mul(t_a[:, dsl], t_a[:, dsl], t_tri[:])
                a_tiles[(kc, h)] = t_a

            def emit_acc(kc, h):
                roff = 0 if kc < 4 * qt else (kc - 4 * qt) * 128
                nsl = slice(roff, _TW)
                hsl = slice(h * 128, (h + 1) * 128)
                t_a = a_tiles.pop((kc, h))
                nc.tensor.matmul(p_den[32 * h:32 * h + 1, nsl], t_onescol,
                                 t_a[:, nsl],
                                 start=(kc == 0), stop=(kc == nchunk - 1))
                nc.tensor.matmul(p_os[h][:, nsl], t_v[kc][:, hsl], t_a[:, nsl],
                                 start=(kc == 0), stop=(kc == nchunk - 1))

            t_zs = [None] * _HPG

            def emit_z(h):
                t_rd = pg.tile([1, _TW], f32, tag="rd", name="rd", bufs=3)
                nc.vector.reciprocal(t_rd[:], p_den[32 * h:32 * h + 1, :])
                t_rdb = pg.tile([128, _TW], f32, tag="rdb", name="rdb", bufs=3)
                nc.gpsimd.partition_broadcast(t_rdb[:], t_rd[:])
                t_z = pg.tile([128, _TW], bf16, tag=f"z{h}", name=f"z{h}", bufs=2)
                nc.vector.tensor_mul(t_z[:], p_os[h][:], t_rdb[:])
                t_zs[h] = t_z

            # heads skewed by SKEW chunks so they finish staggered: each
            # head's z-chain (recip->bcast->mul, ~2.5us) runs while later
            # heads still stream chunks, freeing p_o banks incrementally.
            SKEW = 4
            for v in range(nchunk + 2 * SKEW + LOOKAHEAD + 1):
                for h in range(_HPG):
                    kc_s = v - SKEW * h
                    if 0 <= kc_s < nchunk:
                        emit_s(kc_s, h)
                    kc_a = v - SKEW * h - LOOKAHEAD
                    if 0 <= kc_a < nchunk:
                        emit_acc(kc_a, h)
                        if kc_a == nchunk - 1:
                            emit_z(h)
            return t_zs

        def c_proj(qt, t_zs):
            for tb in range(4):
                bsl = slice(tb * 128, (tb + 1) * 128)
                r0 = qt * _TW + tb * 128
                t_ob = pg.tile([128, _C], bf16, tag="ob", name="ob", bufs=3)
                for nh in range(2):
                    osl = slice(nh * 384, (nh + 1) * 384)
                    p_c = ppj.tile([128, 384], f32, tag="pj", name=f"pc{qt}{tb}{nh}",
                                   padded_shape=[128, _TW])
                    for c in range(_HPG):
                        nc.tensor.matmul(p_c[:], t_zs[c][:, bsl], t_wo[c][:, osl],
                                         start=(c == 0), stop=(c == _HPG - 1))
                    if qt == _NT - 1 and nh == 1:
                        nc.scalar.copy(t_ob[:, osl], p_c[:])
                    else:
                        nc.vector.tensor_copy(t_ob[:, osl], p_c[:])
                eng = nc.sync if tb % 2 == 0 else nc.scalar
                eng.dma_start(out[r0:r0 + 128, :], t_ob[:])

        # ---------------- emission ----------------
        for i in range(_NT):
            k_sqs = [k_chain_a(i, h) for h in range(_HPG)]
            for h in range(_HPG):
                q_chain(i, h)
            for tb in range(4 * i, 4 * i + 4):
                v_group(tb)
            for h in range(_HPG):
                k_chain_b(i, h, k_sqs[h])

        for qt in range(_NT):
            t_zs = attention(qt)
            c_proj(qt, t_zs)

    nc.compile()
    return nc


def _get_nc():
    if "nc" not in _cached:
        _cached["nc"] = _build_nc()
    return _cached["nc"]


def make_in_maps(x, cos, sin, Wq, Wk, Wv, Wo):
    import ml_dtypes
    bf = ml_dtypes.bfloat16
    cosT = np.ascontiguousarray(cos.reshape(_T, _D // 2).T)  # (64, T)
    sinT = np.ascontiguousarray(sin.reshape(_T, _D // 2).T)
    ccm = np.concatenate([cosT, cosT], axis=0).astype(bf)     # (128, T)
    ssm = np.concatenate([sinT, -sinT], axis=0).astype(bf)
    ssrm = np.concatenate([-sinT, sinT], axis=0).astype(bf)
    trim = (np.arange(128)[None, :] >= np.arange(128)[:, None]).astype(bf)
    ones128 = np.ones((128, 128), dtype=bf)
    in_maps = []
    for core in range(8):
        b, g = divmod(core, 2)
        gsl = slice(g * _HD, (g + 1) * _HD)
        in_maps.append({
            "xT": np.ascontiguousarray(x[b].T).astype(bf),
            "wq": np.ascontiguousarray(Wq[gsl, :].T).astype(bf),
            "wk": np.ascontiguousarray(Wk[gsl, :].T).astype(bf),
            "wv": np.ascontiguousarray(Wv[gsl, :].T).astype(bf),
            "wo": np.ascontiguousarray(Wo[:, gsl].T).astype(bf),
            "cc": ccm, "ssr": ssrm, "tri": trim, "onesb": ones128,
        })
    return in_maps


def kernel(x, cos, sin, Wq, Wk, Wv, Wo):
    from concourse.bass_utils import run_bass_kernel_spmd

    x = np.asarray(x, dtype=np.float32)
    cos = np.asarray(cos, dtype=np.float32)
    sin = np.asarray(sin, dtype=np.float32)
    Wq = np.asarray(Wq, dtype=np.float32)
    Wk = np.asarray(Wk, dtype=np.float32)
    Wv = np.asarray(Wv, dtype=np.float32)
    Wo = np.asarray(Wo, dtype=np.float32)

    nc = _get_nc()
    in_maps = make_in_maps(x, cos, sin, Wq, Wk, Wv, Wo)
    res = run_bass_kernel_spmd(nc, in_maps, core_ids=list(range(8)))
    outs = [np.asarray(r_["out"], dtype=np.float32) for r_ in res.results]
    return np.stack([outs[2 * b] + outs[2 * b + 1] for b in range(_B)], axis=0)


# revision 5
# speedup vs baseline: 1.9516x; 1.0025x over previous
"""Trainium2 Bass kernel for CausalSelfAttention (B=4, T=2048, C=768, H=6, D=128)
with RoPE + QK-RMSNorm.  v3: one act-table set, soft-pipelined phases.

Sharding: 8 cores = batch(4) x head-group(2, 3 heads each).

Key points:
  - Q/K/A/V tiles bf16 (DVE 2x, same PE rate); projections/c_proj f32r.
  - RoPE half-swap via partition-offset DVE muls (no PE perm matmul).
  - K-side RMSNorm never scales K: rk = 1/sqrt(ms_k) columns (with 1/sqrt(D)
    folded in) feed exp's per-partition scale.  ms_k columns computed directly
    via matmul(lhsT=sq_chunk, rhs=ones_col) at psum partitions.
  - Q-side RMSNorm via gpsimd partition_all_reduce.
  - rsqrt = exp(-0.5*ln(x)) on Act; every activation func lives in act-table
    set 'natural_log_exp_and_others', pinned via a filtered table view, so
    the scheduler can interleave norm chains with attention exps freely with
    zero table reloads.
  - Softmax den rows for 3 heads pack one PSUM bank at partitions 0/32/64.
"""

import numpy as np

_B, _T, _C, _H, _D = 4, 2048, 768, 6, 128
_HPG = 3            # heads per group (per core)
_HD = _HPG * _D     # 384
_NT = 4             # T tiles of 512
_TW = 512
_NKC = _T // 128    # 16 k-chunks
_NCB = _C // 128    # 6 contraction chunks
_EPS = 1e-15

_cached = {}


def _patch_act_tables():
    """Pin every activation func we use to the 'natural_log_exp_and_others'
    table: present a filtered view to Bacc's table-load pass in which a func
    appears in a non-6 set only if set 6 cannot serve it.  Runtime behavior is
    unchanged (the real set 6 does contain ln/exp/copy/square/identity); this
    only steers load placement so ln/exp alternation never reloads."""
    import concourse.bacc as bacc_mod
    import concourse.hw_specs as hw_mod
    if getattr(bacc_mod, "_act_tables_patched", False):
        return
    orig = hw_mod.get_activation_tables

    def patched(arch):
        tables = orig(arch)
        items = list(tables.items())
        target = None
        for name, funcs in items:
            if name == "natural_log_exp_and_others":
                target = funcs
        if target is None:
            return tables
        out = {}
        for name, funcs in items:
            if name == "natural_log_exp_and_others":
                out[name] = funcs
            else:
                out[name] = {f for f in funcs if f not in target}
        return out

    bacc_mod.get_activation_tables = patched
    bacc_mod._act_tables_patched = True


def _build_nc():
    from contextlib import ExitStack
    from concourse import bacc, tile, mybir, bass_isa

    _patch_act_tables()

    f32 = mybir.dt.float32
    f32r = mybir.dt.float32r
    bf16 = mybir.dt.bfloat16
    Act = mybir.ActivationFunctionType

    nc = bacc.Bacc("TRN2", target_bir_lowering=False, debug=False)

    xT = nc.dram_tensor("xT", (_C, _T), bf16, kind="ExternalInput").ap()
    wq = nc.dram_tensor("wq", (_C, _HD), bf16, kind="ExternalInput").ap()
    wk = nc.dram_tensor("wk", (_C, _HD), bf16, kind="ExternalInput").ap()
    wv = nc.dram_tensor("wv", (_C, _HD), bf16, kind="ExternalInput").ap()
    wo = nc.dram_tensor("wo", (_HD, _C), bf16, kind="ExternalInput").ap()
    cc = nc.dram_tensor("cc", (128, _T), bf16, kind="ExternalInput").ap()
    ssr = nc.dram_tensor("ssr", (128, _T), bf16, kind="ExternalInput").ap()
    tri = nc.dram_tensor("tri", (128, 128), bf16, kind="ExternalInput").ap()
    onesb = nc.dram_tensor("onesb", (128, 128), bf16, kind="ExternalInput").ap()
    out = nc.dram_tensor("out", (_T, _C), bf16, kind="ExternalOutput").ap()

    with tile.TileContext(nc) as tc, ExitStack() as ctx, \
            nc.allow_low_precision(reason="bf16 attention pipeline, f32 accum"):
        # ---------------- pools ----------------
        pc = ctx.enter_context(tc.tile_pool(name="pc", bufs=1))    # persistent
        pg = ctx.enter_context(tc.tile_pool(name="pg", bufs=2))    # scratch
        pa = ctx.enter_context(tc.tile_pool(name="pa", bufs=2))    # A tiles
        # PSUM: 2 + 2 + 3 + 1 = 8 banks
        ppj = ctx.enter_context(tc.tile_pool(name="ppj", bufs=2, space="PSUM"))
        pps = ctx.enter_context(tc.tile_pool(name="pps", bufs=2, space="PSUM"))
        ppo = ctx.enter_context(tc.tile_pool(name="ppo", bufs=3, space="PSUM"))
        ppd = ctx.enter_context(tc.tile_pool(name="ppd", bufs=1, space="PSUM"))

        # ---------------- persistent SBUF tiles + loads ----------------
        # single-DMA weight loads (3D APs) so the SP ring's fixed per-DMA
        # cost never gates the first projection groups.
        t_wkall = pc.tile([128, _NCB * _HD], bf16, tag="wkall")
        t_wqall = pc.tile([128, _NCB * _HD], bf16, tag="wqall")
        t_wvall = pc.tile([128, _NCB * _HD], bf16, tag="wvall")
        t_woall = pc.tile([128, _HPG * _C], bf16, tag="woall")
        t_wk = [t_wkall[:, c * _HD:(c + 1) * _HD] for c in range(_NCB)]
        t_wq = [t_wqall[:, c * _HD:(c + 1) * _HD] for c in range(_NCB)]
        t_wv = [t_wvall[:, c * _HD:(c + 1) * _HD] for c in range(_NCB)]
        t_wo = [t_woall[:, c * _C:(c + 1) * _C] for c in range(_HPG)]

        def _wload(tile_, dram, nchunks, width):
            nc.sync.dma_start(
                tile_[:].rearrange("p (c f) -> p c f", c=nchunks),
                dram.rearrange("(c p) f -> p c f", p=128))

        t_xt = [pc.tile([128, _T], bf16, tag=f"xt{c}", name=f"xt{c}")
                for c in range(_NCB)]
        t_cc = pc.tile([128, _T], bf16, tag="cc")
        t_ssr = pc.tile([128, _T], bf16, tag="ssr")
        t_tri = pc.tile([128, 128], bf16, tag="tri")
        t_ones = pc.tile([128, 128], bf16, tag="ones")

        def _xt_load(i):
            isl = slice(i * _TW, (i + 1) * _TW)
            for c in range(_NCB):
                nc.sync.dma_start(t_xt[c][:, isl], xT[c * 128:(c + 1) * 128, isl])

        _wload(t_wkall, wk, _NCB, _HD)
        _xt_load(0)
        _wload(t_wqall, wq, _NCB, _HD)
        nc.sync.dma_start(t_cc[:], cc[:])
        nc.sync.dma_start(t_ssr[:], ssr[:])
        nc.sync.dma_start(t_ones[:], onesb[:])
        _xt_load(1)
        _wload(t_wvall, wv, _NCB, _HD)
        _xt_load(2)
        nc.sync.dma_start(t_tri[:], tri[:])
        _wload(t_woall, wo, _HPG, _C)
        _xt_load(3)

        t_onescol = t_ones[:, 0:1]
        t_eps = pc.tile([128, 1], f32, tag="eps")
        nc.gpsimd.memset(t_eps[:], _EPS)
        # PE warmup during initial DMA wait: ramps the pstate clock and
        # fills otherwise-idle cycles; lands in the den bank (idle till qt0)
        t_warm = pc.tile([128, _TW], bf16, tag="warm")
        nc.gpsimd.memset(t_warm[:], 0.0)
        p_warm = ppd.tile([128, _TW], f32, tag="pd", name="warm")
        for _w in range(5):
            nc.tensor.matmul(p_warm[:], t_warm[:, 0:128], t_warm[:],
                             start=True, stop=True)

        # persistent per-head tensors
        t_kn = [pc.tile([128, _T], bf16, tag=f"kn{h}", name=f"kn{h}")
                for h in range(_HPG)]
        t_q = [pc.tile([128, _T], bf16, tag=f"q{h}", name=f"q{h}")
               for h in range(_HPG)]
        t_v = [pc.tile([128, _HD], bf16, tag=f"v{tb}", name=f"v{tb}")
               for tb in range(_NKC)]
        t_rk = [pc.tile([128, _NKC], f32, tag=f"rk{h}", name=f"rk{h}")
                for h in range(_HPG)]

        # ================ projections + rope + norms ================
        def proj_group(ws, h, isl, tag, halves=False):
            hsl = slice(h * 128, (h + 1) * 128)
            p = ppj.tile([128, _TW], f32, tag="pj", name=f"p{tag}")
            if halves:
                for colsl in (slice(0, 256), slice(256, 512)):
                    for c in range(_NCB):
                        nc.tensor.matmul(p[:, colsl], ws[c][:, hsl],
                                         t_xt[c][:, isl][:, colsl],
                                         start=(c == 0), stop=(c == _NCB - 1))
            else:
                for c in range(_NCB):
                    nc.tensor.matmul(p[:], ws[c][:, hsl], t_xt[c][:, isl],
                                     start=(c == 0), stop=(c == _NCB - 1))
            return p

        def rope_sq(dst_slice, p_raw, i, add_on_pool, evac_on_act):
            """dst = raw*cc + swap(raw)*ss (bf16); returns bf16 squares tile."""
            isl = slice(i * _TW, (i + 1) * _TW)
            t_raw = pg.tile([128, _TW], bf16, tag="raw", name="raw", bufs=4)
            if evac_on_act:
                nc.scalar.copy(t_raw[:], p_raw[:])
            else:
                nc.vector.tensor_copy(t_raw[:], p_raw[:])
            t_swm = pg.tile([128, _TW], bf16, tag="swm", name="swm", bufs=4)
            nc.vector.tensor_mul(t_swm[0:64, :], t_raw[64:128, :], t_ssr[64:128, isl])
            nc.vector.tensor_mul(t_swm[64:128, :], t_raw[0:64, :], t_ssr[0:64, isl])
            nc.vector.tensor_mul(dst_slice, t_raw[:], t_cc[:, isl])
            if add_on_pool:
                nc.gpsimd.tensor_add(dst_slice, dst_slice, t_swm[:])
            else:
                nc.vector.tensor_add(dst_slice, dst_slice, t_swm[:])
            t_sq = pg.tile([128, _TW], bf16, tag="sq", name="sq", bufs=4)
            if add_on_pool:
                nc.scalar.square(t_sq[:], dst_slice)
            else:
                nc.vector.tensor_mul(t_sq[:], dst_slice, dst_slice)
            return t_sq

        def k_chain_a(i, h):
            isl = slice(i * _TW, (i + 1) * _TW)
            p_k = proj_group(t_wk, h, isl, f"k{i}{h}")
            return rope_sq(t_kn[h][:, isl], p_k, i, add_on_pool=True, evac_on_act=True)

        def k_chain_b(i, h, t_sq):
            # ms_k columns (128,4) then rk = exp(-0.5*ln(ms_k))
            p_cols = pps.tile([128, 4], f32, tag="ps", name=f"cols{i}{h}",
                              padded_shape=[128, _TW])
            for j in range(4):
                nc.tensor.matmul(p_cols[:, j:j + 1], t_sq[:, j * 128:(j + 1) * 128],
                                 t_onescol, start=True, stop=True)
            t_lnk = pg.tile([128, 4], f32, tag="lnk", name="lnk", bufs=4)
            nc.scalar.activation(t_lnk[:], p_cols[:], Act.Ln,
                                 bias=t_eps[:], scale=1.0)
            nc.scalar.activation(t_rk[h][:, 4 * i:4 * i + 4], t_lnk[:], Act.Exp,
                                 bias=0.0, scale=-0.5)

        def q_chain(i, h):
            isl = slice(i * _TW, (i + 1) * _TW)
            p_q = proj_group(t_wq, h, isl, f"q{i}{h}")
            t_sq = rope_sq(t_q[h][:, isl], p_q, i, add_on_pool=False, evac_on_act=False)
            t_ms = pg.tile([128, _TW], bf16, tag="ms", name="ms", bufs=2)
            nc.gpsimd.partition_all_reduce(t_ms[:], t_sq[:], channels=128,
                                           reduce_op=bass_isa.ReduceOp.add)
            t_ln = pg.tile([128, _TW], bf16, tag="qln", name="qln", bufs=2)
            nc.scalar.activation(t_ln[:], t_ms[:], Act.Ln,
                                 bias=t_eps[:], scale=1.0 / 128.0)
            t_r = pg.tile([128, _TW], bf16, tag="qr", name="qr", bufs=2)
            nc.scalar.activation(t_r[:], t_ln[:], Act.Exp, bias=0.0, scale=-0.5)
            nc.vector.tensor_mul(t_q[h][:, isl], t_q[h][:, isl], t_r[:])

        def v_group(tb):
            bsl = slice(tb * 128, (tb + 1) * 128)
            p_v = ppj.tile([128, _HD], f32, tag="pj", name=f"pv{tb}",
                           padded_shape=[128, _TW])
            for c in range(_NCB):
                nc.tensor.matmul(p_v[:], t_xt[c][:, bsl], t_wv[c][:],
                                 start=(c == 0), stop=(c == _NCB - 1))
            if tb % 2 == 0:
                nc.vector.tensor_copy(t_v[tb][:], p_v[:])
            else:
                nc.scalar.copy(t_v[tb][:], p_v[:])

        # ================ attention + c_proj ================
        def attention(qt):
            nchunk = 4 * qt + 4
            LOOKAHEAD = 3
            qsl = slice(qt * _TW, (qt + 1) * _TW)
            p_den = ppd.tile([128, _TW], f32, tag="pd", name=f"pd{qt}")
            p_os = [ppo.tile([128, _TW], f32, tag="po", name=f"po{qt}{h}")
                    for h in range(_HPG)]
            a_tiles = {}

            def emit_s(kc, h):
                roff = 0 if kc < 4 * qt else (kc - 4 * qt) * 128
                nsl = slice(roff, _TW)
                ksl = slice(kc * 128, (kc + 1) * 128)
                p_s = pps.tile([128, _TW], f32, tag="ps", name=f"s{kc}{h}")
                nc.tensor.matmul(p_s[:, nsl], t_kn[h][:, ksl],
                                 t_q[h][:, qsl][:, nsl], start=True, stop=True)
                t_a = pa.tile([128, _TW], bf16, tag="a", name="a", bufs=32)
                nc.scalar.activation(t_a[:, nsl], p_s[:, nsl], Act.Exp,
                                     bias=0.0, scale=t_rk[h][:, kc:kc + 1])
                if kc >= 4 * qt:
                    dsl = slice(roff, roff + 128)
                    nc.vector.tensor_mul(t_a[:, dsl], t_a[:, dsl], t_tri[:])
                a_tiles[(kc, h)] = t_a

            def emit_acc(kc, h):
                roff = 0 if kc < 4 * qt else (kc - 4 * qt) * 128
                nsl = slice(roff, _TW)
                hsl = slice(h * 128, (h + 1) * 128)
                t_a = a_tiles.pop((kc, h))
                nc.tensor.matmul(p_den[32 * h:32 * h + 1, nsl], t_onescol,
                                 t_a[:, nsl],
                                 start=(kc == 0), stop=(kc == nchunk - 1))
                nc.tensor.matmul(p_os[h][:, nsl], t_v[kc][:, hsl], t_a[:, nsl],
                                 start=(kc == 0), stop=(kc == nchunk - 1))

            t_zs = [None] * _HPG

            def emit_z(h):
                t_rd = pg.tile([1, _TW], f32, tag="rd", name="rd", bufs=3)
                nc.vector.reciprocal(t_rd[:], p_den[32 * h:32 * h + 1, :])
                t_rdb = pg.tile([128, _TW], f32, tag="rdb", name="rdb", bufs=3)
                nc.gpsimd.partition_broadcast(t_rdb[:], t_rd[:])
                t_z = pg.tile([128, _TW], bf16, tag=f"z{h}", name=f"z{h}", bufs=2)
                nc.vector.tensor_mul(t_z[:], p_os[h][:], t_rdb[:])
                t_zs[h] = t_z

            # heads skewed by SKEW chunks so they finish staggered: each
            # head's z-chain (recip->bcast->mul, ~2.5us) runs while later
            # heads still stream chunks, freeing p_o banks incrementally.
            SKEW = 4
            for v in range(nchunk + 2 * SKEW + LOOKAHEAD + 1):
                for h in range(_HPG):
                    kc_s = v - SKEW * h
                    if 0 <= kc_s < nchunk:
                        emit_s(kc_s, h)
                    kc_a = v - SKEW * h - LOOKAHEAD
                    if 0 <= kc_a < nchunk:
                        emit_acc(kc_a, h)
                        if kc_a == nchunk - 1:
                            emit_z(h)
            return t_zs

        def c_proj(qt, t_zs):
            for tb in range(4):
                bsl = slice(tb * 128, (tb + 1) * 128)
                r0 = qt * _TW + tb * 128
                t_ob = pg.tile([128, _C], bf16, tag="ob", name="ob", bufs=3)
                for nh in range(2):
                    osl = slice(nh * 384, (nh + 1) * 384)
                    p_c = ppj.tile([128, 384], f32, tag="pj", name=f"pc{qt}{tb}{nh}",
                                   padded_shape=[128, _TW])
                    for c in range(_HPG):
                        nc.tensor.matmul(p_c[:], t_zs[c][:, bsl], t_wo[c][:, osl],
                                         start=(c == 0), stop=(c == _HPG - 1))
                    if qt == _NT - 1 and nh == 1:
                        nc.scalar.copy(t_ob[:, osl], p_c[:])
                    else:
                        nc.vector.tensor_copy(t_ob[:, osl], p_c[:])
                eng = nc.sync if tb % 2 == 0 else nc.scalar
                eng.dma_start(out[r0:r0 + 128, :], t_ob[:])

        # ---------------- emission ----------------
        for i in range(_NT):
            k_sqs = [k_chain_a(i, h) for h in range(_HPG)]
            for h in range(_HPG):
                q_chain(i, h)
            for tb in range(4 * i, 4 * i + 4):
                v_group(tb)
            for h in range(_HPG):
                k_chain_b(i, h, k_sqs[h])

        for qt in range(_NT):
            t_zs = attention(qt)
            c_proj(qt, t_zs)

    nc.compile()
    return nc


def _get_nc():
    if "nc" not in _cached:
        _cached["nc"] = _build_nc()
    return _cached["nc"]


def make_in_maps(x, cos, sin, Wq, Wk, Wv, Wo):
    import ml_dtypes
    bf = ml_dtypes.bfloat16
    cosT = np.ascontiguousarray(cos.reshape(_T, _D // 2).T)  # (64, T)
    sinT = np.ascontiguousarray(sin.reshape(_T, _D // 2).T)
    ccm = np.concatenate([cosT, cosT], axis=0).astype(bf)     # (128, T)
    ssm = np.concatenate([sinT, -sinT], axis=0).astype(bf)
    ssrm = np.concatenate([-sinT, sinT], axis=0).astype(bf)
    trim = (np.arange(128)[None, :] >= np.arange(128)[:, None]).astype(bf)
    ones128 = np.ones((128, 128), dtype=bf)
    in_maps = []
    for core in range(8):
        b, g = divmod(core, 2)
        gsl = slice(g * _HD, (g + 1) * _HD)
        in_maps.append({
            "xT": np.ascontiguousarray(x[b].T).astype(bf),
            "wq": np.ascontiguousarray(Wq[gsl, :].T).astype(bf),
            "wk": np.ascontiguousarray(Wk[gsl, :].T).astype(bf),
            "wv": np.ascontiguousarray(Wv[gsl, :].T).astype(bf),
            "wo": np.ascontiguousarray(Wo[:, gsl].T).astype(bf),
            "cc": ccm, "ssr": ssrm, "tri": trim, "onesb": ones128,
        })
    return in_maps


def kernel(x, cos, sin, Wq, Wk, Wv, Wo):
    from concourse.bass_utils import run_bass_kernel_spmd

    x = np.asarray(x, dtype=np.float32)
    cos = np.asarray(cos, dtype=np.float32)
    sin = np.asarray(sin, dtype=np.float32)
    Wq = np.asarray(Wq, dtype=np.float32)
    Wk = np.asarray(Wk, dtype=np.float32)
    Wv = np.asarray(Wv, dtype=np.float32)
    Wo = np.asarray(Wo, dtype=np.float32)

    nc = _get_nc()
    in_maps = make_in_maps(x, cos, sin, Wq, Wk, Wv, Wo)
    res = run_bass_kernel_spmd(nc, in_maps, core_ids=list(range(8)))
    outs = [np.asarray(r_["out"], dtype=np.float32) for r_ in res.results]
    return np.stack([outs[2 * b] + outs[2 * b + 1] for b in range(_B)], axis=0)


# revision 6
# speedup vs baseline: 1.9551x; 1.0018x over previous
"""Trainium2 Bass kernel for CausalSelfAttention (B=4, T=2048, C=768, H=6, D=128)
with RoPE + QK-RMSNorm.  v3: one act-table set, soft-pipelined phases.

Sharding: 8 cores = batch(4) x head-group(2, 3 heads each).

Key points:
  - Q/K/A/V tiles bf16 (DVE 2x, same PE rate); projections/c_proj f32r.
  - RoPE half-swap via partition-offset DVE muls (no PE perm matmul).
  - K-side RMSNorm never scales K: rk = 1/sqrt(ms_k) columns (with 1/sqrt(D)
    folded in) feed exp's per-partition scale.  ms_k columns computed directly
    via matmul(lhsT=sq_chunk, rhs=ones_col) at psum partitions.
  - Q-side RMSNorm via gpsimd partition_all_reduce.
  - rsqrt = exp(-0.5*ln(x)) on Act; every activation func lives in act-table
    set 'natural_log_exp_and_others', pinned via a filtered table view, so
    the scheduler can interleave norm chains with attention exps freely with
    zero table reloads.
  - Softmax den rows for 3 heads pack one PSUM bank at partitions 0/32/64.
"""

import numpy as np

_B, _T, _C, _H, _D = 4, 2048, 768, 6, 128
_HPG = 3            # heads per group (per core)
_HD = _HPG * _D     # 384
_NT = 4             # T tiles of 512
_TW = 512
_NKC = _T // 128    # 16 k-chunks
_NCB = _C // 128    # 6 contraction chunks
_EPS = 1e-15

_cached = {}


def _patch_act_tables():
    """Pin every activation func we use to the 'natural_log_exp_and_others'
    table: present a filtered view to Bacc's table-load pass in which a func
    appears in a non-6 set only if set 6 cannot serve it.  Runtime behavior is
    unchanged (the real set 6 does contain ln/exp/copy/square/identity); this
    only steers load placement so ln/exp alternation never reloads."""
    import concourse.bacc as bacc_mod
    import concourse.hw_specs as hw_mod
    if getattr(bacc_mod, "_act_tables_patched", False):
        return
    orig = hw_mod.get_activation_tables

    def patched(arch):
        tables = orig(arch)
        items = list(tables.items())
        target = None
        for name, funcs in items:
            if name == "natural_log_exp_and_others":
                target = funcs
        if target is None:
            return tables
        out = {}
        for name, funcs in items:
            if name == "natural_log_exp_and_others":
                out[name] = funcs
            else:
                out[name] = {f for f in funcs if f not in target}
        return out

    bacc_mod.get_activation_tables = patched
    bacc_mod._act_tables_patched = True


def _build_nc():
    from contextlib import ExitStack
    from concourse import bacc, tile, mybir, bass_isa

    _patch_act_tables()

    f32 = mybir.dt.float32
    f32r = mybir.dt.float32r
    bf16 = mybir.dt.bfloat16
    Act = mybir.ActivationFunctionType

    nc = bacc.Bacc("TRN2", target_bir_lowering=False, debug=False)

    xT = nc.dram_tensor("xT", (_C, _T), bf16, kind="ExternalInput").ap()
    wq = nc.dram_tensor("wq", (_C, _HD), bf16, kind="ExternalInput").ap()
    wk = nc.dram_tensor("wk", (_C, _HD), bf16, kind="ExternalInput").ap()
    wv = nc.dram_tensor("wv", (_C, _HD), bf16, kind="ExternalInput").ap()
    wo = nc.dram_tensor("wo", (_HD, _C), bf16, kind="ExternalInput").ap()
    cc = nc.dram_tensor("cc", (128, _T), bf16, kind="ExternalInput").ap()
    ssr = nc.dram_tensor("ssr", (128, _T), bf16, kind="ExternalInput").ap()
    tri = nc.dram_tensor("tri", (128, 128), bf16, kind="ExternalInput").ap()
    onesb = nc.dram_tensor("onesb", (128, 128), bf16, kind="ExternalInput").ap()
    out = nc.dram_tensor("out", (_T, _C), bf16, kind="ExternalOutput").ap()

    with tile.TileContext(nc) as tc, ExitStack() as ctx, \
            nc.allow_low_precision(reason="bf16 attention pipeline, f32 accum"):
        # ---------------- pools ----------------
        pc = ctx.enter_context(tc.tile_pool(name="pc", bufs=1))    # persistent
        pg = ctx.enter_context(tc.tile_pool(name="pg", bufs=2))    # scratch
        pa = ctx.enter_context(tc.tile_pool(name="pa", bufs=2))    # A tiles
        # PSUM: 2 + 2 + 3 + 1 = 8 banks
        ppj = ctx.enter_context(tc.tile_pool(name="ppj", bufs=2, space="PSUM"))
        pps = ctx.enter_context(tc.tile_pool(name="pps", bufs=2, space="PSUM"))
        ppo = ctx.enter_context(tc.tile_pool(name="ppo", bufs=3, space="PSUM"))
        ppd = ctx.enter_context(tc.tile_pool(name="ppd", bufs=1, space="PSUM"))

        # ---------------- persistent SBUF tiles + loads ----------------
        # single-DMA weight loads (3D APs) so the SP ring's fixed per-DMA
        # cost never gates the first projection groups.
        t_wkall = pc.tile([128, _NCB * _HD], bf16, tag="wkall")
        t_wqall = pc.tile([128, _NCB * _HD], bf16, tag="wqall")
        t_wvall = pc.tile([128, _NCB * _HD], bf16, tag="wvall")
        t_woall = pc.tile([128, _HPG * _C], bf16, tag="woall")
        t_wk = [t_wkall[:, c * _HD:(c + 1) * _HD] for c in range(_NCB)]
        t_wq = [t_wqall[:, c * _HD:(c + 1) * _HD] for c in range(_NCB)]
        t_wv = [t_wvall[:, c * _HD:(c + 1) * _HD] for c in range(_NCB)]
        t_wo = [t_woall[:, c * _C:(c + 1) * _C] for c in range(_HPG)]

        def _wload(tile_, dram, nchunks, width):
            nc.sync.dma_start(
                tile_[:].rearrange("p (c f) -> p c f", c=nchunks),
                dram.rearrange("(c p) f -> p c f", p=128))

        t_xt = [pc.tile([128, _T], bf16, tag=f"xt{c}", name=f"xt{c}")
                for c in range(_NCB)]
        t_cc = pc.tile([128, _T], bf16, tag="cc")
        t_ssr = pc.tile([128, _T], bf16, tag="ssr")
        t_tri = pc.tile([128, 128], bf16, tag="tri")
        t_ones = pc.tile([128, 128], bf16, tag="ones")

        def _xt_load(i):
            isl = slice(i * _TW, (i + 1) * _TW)
            for c in range(_NCB):
                nc.sync.dma_start(t_xt[c][:, isl], xT[c * 128:(c + 1) * 128, isl])

        _wload(t_wkall, wk, _NCB, _HD)
        _xt_load(0)
        _wload(t_wqall, wq, _NCB, _HD)
        nc.sync.dma_start(t_cc[:], cc[:])
        nc.sync.dma_start(t_ssr[:], ssr[:])
        nc.sync.dma_start(t_ones[:], onesb[:])
        _xt_load(1)
        _wload(t_wvall, wv, _NCB, _HD)
        _xt_load(2)
        nc.sync.dma_start(t_tri[:], tri[:])
        _wload(t_woall, wo, _HPG, _C)
        _xt_load(3)

        t_onescol = t_ones[:, 0:1]
        t_eps = pc.tile([128, 1], f32, tag="eps")
        nc.gpsimd.memset(t_eps[:], _EPS)
        # PE warmup during initial DMA wait: ramps the pstate clock and
        # fills otherwise-idle cycles; lands in the den bank (idle till qt0)
        t_warm = pc.tile([128, _TW], bf16, tag="warm")
        nc.gpsimd.memset(t_warm[:], 0.0)
        p_warm = ppd.tile([128, _TW], f32, tag="pd", name="warm")
        for _w in range(5):
            nc.tensor.matmul(p_warm[:], t_warm[:, 0:128], t_warm[:],
                             start=True, stop=True)

        # persistent per-head tensors
        t_kn = [pc.tile([128, _T], bf16, tag=f"kn{h}", name=f"kn{h}")
                for h in range(_HPG)]
        t_q = [pc.tile([128, _T], bf16, tag=f"q{h}", name=f"q{h}")
               for h in range(_HPG)]
        t_v = [pc.tile([128, _HD], bf16, tag=f"v{tb}", name=f"v{tb}")
               for tb in range(_NKC)]
        t_rk = [pc.tile([128, _NKC], f32, tag=f"rk{h}", name=f"rk{h}")
                for h in range(_HPG)]

        # ================ projections + rope + norms ================
        def proj_group(ws, h, isl, tag, halves=False):
            hsl = slice(h * 128, (h + 1) * 128)
            p = ppj.tile([128, _TW], f32, tag="pj", name=f"p{tag}")
            if halves:
                for colsl in (slice(0, 256), slice(256, 512)):
                    for c in range(_NCB):
                        nc.tensor.matmul(p[:, colsl], ws[c][:, hsl],
                                         t_xt[c][:, isl][:, colsl],
                                         start=(c == 0), stop=(c == _NCB - 1))
            else:
                for c in range(_NCB):
                    nc.tensor.matmul(p[:], ws[c][:, hsl], t_xt[c][:, isl],
                                     start=(c == 0), stop=(c == _NCB - 1))
            return p

        def rope_sq(dst_slice, p_raw, i, add_on_pool, evac_on_act):
            """dst = raw*cc + swap(raw)*ss (bf16); returns bf16 squares tile."""
            isl = slice(i * _TW, (i + 1) * _TW)
            t_raw = pg.tile([128, _TW], bf16, tag="raw", name="raw", bufs=8)
            if evac_on_act:
                nc.scalar.copy(t_raw[:], p_raw[:])
            else:
                nc.vector.tensor_copy(t_raw[:], p_raw[:])
            t_swm = pg.tile([128, _TW], bf16, tag="swm", name="swm", bufs=8)
            nc.vector.tensor_mul(t_swm[0:64, :], t_raw[64:128, :], t_ssr[64:128, isl])
            nc.vector.tensor_mul(t_swm[64:128, :], t_raw[0:64, :], t_ssr[0:64, isl])
            nc.vector.tensor_mul(dst_slice, t_raw[:], t_cc[:, isl])
            if add_on_pool:
                nc.gpsimd.tensor_add(dst_slice, dst_slice, t_swm[:])
            else:
                nc.vector.tensor_add(dst_slice, dst_slice, t_swm[:])
            t_sq = pg.tile([128, _TW], bf16, tag="sq", name="sq", bufs=8)
            if add_on_pool:
                nc.scalar.square(t_sq[:], dst_slice)
            else:
                nc.vector.tensor_mul(t_sq[:], dst_slice, dst_slice)
            return t_sq

        def k_chain_a(i, h):
            isl = slice(i * _TW, (i + 1) * _TW)
            p_k = proj_group(t_wk, h, isl, f"k{i}{h}")
            return rope_sq(t_kn[h][:, isl], p_k, i, add_on_pool=True, evac_on_act=True)

        def k_chain_b(i, h, t_sq):
            # ms_k columns (128,4) then rk = exp(-0.5*ln(ms_k))
            p_cols = pps.tile([128, 4], f32, tag="ps", name=f"cols{i}{h}",
                              padded_shape=[128, _TW])
            for j in range(4):
                nc.tensor.matmul(p_cols[:, j:j + 1], t_sq[:, j * 128:(j + 1) * 128],
                                 t_onescol, start=True, stop=True)
            t_lnk = pg.tile([128, 4], f32, tag="lnk", name="lnk", bufs=4)
            nc.scalar.activation(t_lnk[:], p_cols[:], Act.Ln,
                                 bias=t_eps[:], scale=1.0)
            nc.scalar.activation(t_rk[h][:, 4 * i:4 * i + 4], t_lnk[:], Act.Exp,
                                 bias=0.0, scale=-0.5)

        def q_chain(i, h):
            isl = slice(i * _TW, (i + 1) * _TW)
            p_q = proj_group(t_wq, h, isl, f"q{i}{h}")
            t_sq = rope_sq(t_q[h][:, isl], p_q, i, add_on_pool=False, evac_on_act=False)
            t_ms = pg.tile([128, _TW], bf16, tag="ms", name="ms", bufs=4)
            nc.gpsimd.partition_all_reduce(t_ms[:], t_sq[:], channels=128,
                                           reduce_op=bass_isa.ReduceOp.add)
            t_ln = pg.tile([128, _TW], bf16, tag="qln", name="qln", bufs=4)
            nc.scalar.activation(t_ln[:], t_ms[:], Act.Ln,
                                 bias=t_eps[:], scale=1.0 / 128.0)
            t_r = pg.tile([128, _TW], bf16, tag="qr", name="qr", bufs=4)
            nc.scalar.activation(t_r[:], t_ln[:], Act.Exp, bias=0.0, scale=-0.5)
            nc.vector.tensor_mul(t_q[h][:, isl], t_q[h][:, isl], t_r[:])

        def v_group(tb):
            bsl = slice(tb * 128, (tb + 1) * 128)
            p_v = ppj.tile([128, _HD], f32, tag="pj", name=f"pv{tb}",
                           padded_shape=[128, _TW])
            for c in range(_NCB):
                nc.tensor.matmul(p_v[:], t_xt[c][:, bsl], t_wv[c][:],
                                 start=(c == 0), stop=(c == _NCB - 1))
            if tb % 2 == 0:
                nc.vector.tensor_copy(t_v[tb][:], p_v[:])
            else:
                nc.scalar.copy(t_v[tb][:], p_v[:])

        # ================ attention + c_proj ================
        def attention(qt):
            nchunk = 4 * qt + 4
            LOOKAHEAD = 3
            qsl = slice(qt * _TW, (qt + 1) * _TW)
            p_den = ppd.tile([128, _TW], f32, tag="pd", name=f"pd{qt}")
            p_os = [ppo.tile([128, _TW], f32, tag="po", name=f"po{qt}{h}")
                    for h in range(_HPG)]
            a_tiles = {}

            def emit_s(kc, h):
                roff = 0 if kc < 4 * qt else (kc - 4 * qt) * 128
                nsl = slice(roff, _TW)
                ksl = slice(kc * 128, (kc + 1) * 128)
                p_s = pps.tile([128, _TW], f32, tag="ps", name=f"s{kc}{h}")
                nc.tensor.matmul(p_s[:, nsl], t_kn[h][:, ksl],
                                 t_q[h][:, qsl][:, nsl], start=True, stop=True)
                t_a = pa.tile([128, _TW], bf16, tag="a", name="a", bufs=32)
                nc.scalar.activation(t_a[:, nsl], p_s[:, nsl], Act.Exp,
                                     bias=0.0, scale=t_rk[h][:, kc:kc + 1])
                if kc >= 4 * qt:
                    dsl = slice(roff, roff + 128)
                    nc.vector.tensor_mul(t_a[:, dsl], t_a[:, dsl], t_tri[:])
                a_tiles[(kc, h)] = t_a

            def emit_acc(kc, h):
                roff = 0 if kc < 4 * qt else (kc - 4 * qt) * 128
                nsl = slice(roff, _TW)
                hsl = slice(h * 128, (h + 1) * 128)
                t_a = a_tiles.pop((kc, h))
                nc.tensor.matmul(p_den[32 * h:32 * h + 1, nsl], t_onescol,
                                 t_a[:, nsl],
                                 start=(kc == 0), stop=(kc == nchunk - 1))
                nc.tensor.matmul(p_os[h][:, nsl], t_v[kc][:, hsl], t_a[:, nsl],
                                 start=(kc == 0), stop=(kc == nchunk - 1))

            t_zs = [None] * _HPG

            def emit_z(h):
                t_rd = pg.tile([1, _TW], f32, tag="rd", name="rd", bufs=4)
                nc.vector.reciprocal(t_rd[:], p_den[32 * h:32 * h + 1, :])
                t_rdb = pg.tile([128, _TW], f32, tag="rdb", name="rdb", bufs=4)
                nc.gpsimd.partition_broadcast(t_rdb[:], t_rd[:])
                t_z = pg.tile([128, _TW], bf16, tag=f"z{h}", name=f"z{h}", bufs=2)
                nc.vector.tensor_mul(t_z[:], p_os[h][:], t_rdb[:])
                t_zs[h] = t_z

            # heads skewed by SKEW chunks so they finish staggered: each
            # head's z-chain (recip->bcast->mul, ~2.5us) runs while later
            # heads still stream chunks, freeing p_o banks incrementally.
            SKEW = 4
            for v in range(nchunk + 2 * SKEW + LOOKAHEAD + 1):
                for h in range(_HPG):
                    kc_s = v - SKEW * h
                    if 0 <= kc_s < nchunk:
                        emit_s(kc_s, h)
                    kc_a = v - SKEW * h - LOOKAHEAD
                    if 0 <= kc_a < nchunk:
                        emit_acc(kc_a, h)
                        if kc_a == nchunk - 1:
                            emit_z(h)
            return t_zs

        def c_proj(qt, t_zs):
            for tb in range(4):
                bsl = slice(tb * 128, (tb + 1) * 128)
                r0 = qt * _TW + tb * 128
                t_ob = pg.tile([128, _C], bf16, tag="ob", name="ob", bufs=3)
                for nh in range(2):
                    osl = slice(nh * 384, (nh + 1) * 384)
                    p_c = ppj.tile([128, 384], f32, tag="pj", name=f"pc{qt}{tb}{nh}",
                                   padded_shape=[128, _TW])
                    for c in range(_HPG):
                        nc.tensor.matmul(p_c[:], t_zs[c][:, bsl], t_wo[c][:, osl],
                                         start=(c == 0), stop=(c == _HPG - 1))
                    if qt == _NT - 1 and nh == 1:
                        nc.scalar.copy(t_ob[:, osl], p_c[:])
                    else:
                        nc.vector.tensor_copy(t_ob[:, osl], p_c[:])
                eng = nc.sync if tb % 2 == 0 else nc.scalar
                eng.dma_start(out[r0:r0 + 128, :], t_ob[:])

        # ---------------- emission ----------------
        for i in range(_NT):
            k_sqs = [k_chain_a(i, h) for h in range(_HPG)]
            for h in range(_HPG):
                q_chain(i, h)
            for tb in range(4 * i, 4 * i + 4):
                v_group(tb)
            for h in range(_HPG):
                k_chain_b(i, h, k_sqs[h])

        for qt in range(_NT):
            t_zs = attention(qt)
            c_proj(qt, t_zs)

    nc.compile()
    return nc


def _get_nc():
    if "nc" not in _cached:
        _cached["nc"] = _build_nc()
    return _cached["nc"]


def make_in_maps(x, cos, sin, Wq, Wk, Wv, Wo):
    import ml_dtypes
    bf = ml_dtypes.bfloat16
    cosT = np.ascontiguousarray(cos.reshape(_T, _D // 2).T)  # (64, T)
    sinT = np.ascontiguousarray(sin.reshape(_T, _D // 2).T)
    ccm = np.concatenate([cosT, cosT], axis=0).astype(bf)     # (128, T)
    ssm = np.concatenate([sinT, -sinT], axis=0).astype(bf)
    ssrm = np.concatenate([-sinT, sinT], axis=0).astype(bf)
    trim = (np.arange(128)[None, :] >= np.arange(128)[:, None]).astype(bf)
    ones128 = np.ones((128, 128), dtype=bf)
    in_maps = []
    for core in range(8):
        b, g = divmod(core, 2)
        gsl = slice(g * _HD, (g + 1) * _HD)
        in_maps.append({
            "xT": np.ascontiguousarray(x[b].T).astype(bf),
            "wq": np.ascontiguousarray(Wq[gsl, :].T).astype(bf),
            "wk": np.ascontiguousarray(Wk[gsl, :].T).astype(bf),
            "wv": np.ascontiguousarray(Wv[gsl, :].T).astype(bf),
            "wo": np.ascontiguousarray(Wo[:, gsl].T).astype(bf),
            "cc": ccm, "ssr": ssrm, "tri": trim, "onesb": ones128,
        })
    return in_maps


def kernel(x, cos, sin, Wq, Wk, Wv, Wo):
    from concourse.bass_utils import run_bass_kernel_spmd

    x = np.asarray(x, dtype=np.float32)
    cos = np.asarray(cos, dtype=np.float32)
    sin = np.asarray(sin, dtype=np.float32)
    Wq = np.asarray(Wq, dtype=np.float32)
    Wk = np.asarray(Wk, dtype=np.float32)
    Wv = np.asarray(Wv, dtype=np.float32)
    Wo = np.asarray(Wo, dtype=np.float32)

    nc = _get_nc()
    in_maps = make_in_maps(x, cos, sin, Wq, Wk, Wv, Wo)
    res = run_bass_kernel_spmd(nc, in_maps, core_ids=list(range(8)))
    outs = [np.asarray(r_["out"], dtype=np.float32) for r_ in res.results]
    return np.stack([outs[2 * b] + outs[2 * b + 1] for b in range(_B)], axis=0)


# revision 7
# speedup vs baseline: 1.9582x; 1.0015x over previous
"""Trainium2 Bass kernel for CausalSelfAttention (B=4, T=2048, C=768, H=6, D=128)
with RoPE + QK-RMSNorm.  v3: one act-table set, soft-pipelined phases.

Sharding: 8 cores = batch(4) x head-group(2, 3 heads each).

Key points:
  - Q/K/A/V tiles bf16 (DVE 2x, same PE rate); projections/c_proj f32r.
  - RoPE half-swap via partition-offset DVE muls (no PE perm matmul).
  - K-side RMSNorm never scales K: rk = 1/sqrt(ms_k) columns (with 1/sqrt(D)
    folded in) feed exp's per-partition scale.  ms_k columns computed directly
    via matmul(lhsT=sq_chunk, rhs=ones_col) at psum partitions.
  - Q-side RMSNorm via gpsimd partition_all_reduce.
  - rsqrt = exp(-0.5*ln(x)) on Act; every activation func lives in act-table
    set 'natural_log_exp_and_others', pinned via a filtered table view, so
    the scheduler can interleave norm chains with attention exps freely with
    zero table reloads.
  - Softmax den rows for 3 heads pack one PSUM bank at partitions 0/32/64.
"""

import numpy as np

_B, _T, _C, _H, _D = 4, 2048, 768, 6, 128
_HPG = 3            # heads per group (per core)
_HD = _HPG * _D     # 384
_NT = 4             # T tiles of 512
_TW = 512
_NKC = _T // 128    # 16 k-chunks
_NCB = _C // 128    # 6 contraction chunks
_EPS = 1e-15

_cached = {}


def _patch_act_tables():
    """Pin every activation func we use to the 'natural_log_exp_and_others'
    table: present a filtered view to Bacc's table-load pass in which a func
    appears in a non-6 set only if set 6 cannot serve it.  Runtime behavior is
    unchanged (the real set 6 does contain ln/exp/copy/square/identity); this
    only steers load placement so ln/exp alternation never reloads."""
    import concourse.bacc as bacc_mod
    import concourse.hw_specs as hw_mod
    if getattr(bacc_mod, "_act_tables_patched", False):
        return
    orig = hw_mod.get_activation_tables

    def patched(arch):
        tables = orig(arch)
        items = list(tables.items())
        target = None
        for name, funcs in items:
            if name == "natural_log_exp_and_others":
                target = funcs
        if target is None:
            return tables
        out = {}
        for name, funcs in items:
            if name == "natural_log_exp_and_others":
                out[name] = funcs
            else:
                out[name] = {f for f in funcs if f not in target}
        return out

    bacc_mod.get_activation_tables = patched
    bacc_mod._act_tables_patched = True


def _build_nc():
    from contextlib import ExitStack
    from concourse import bacc, tile, mybir, bass_isa

    _patch_act_tables()

    f32 = mybir.dt.float32
    f32r = mybir.dt.float32r
    bf16 = mybir.dt.bfloat16
    Act = mybir.ActivationFunctionType

    nc = bacc.Bacc("TRN2", target_bir_lowering=False, debug=False)

    xT = nc.dram_tensor("xT", (_C, _T), bf16, kind="ExternalInput").ap()
    wq = nc.dram_tensor("wq", (_C, _HD), bf16, kind="ExternalInput").ap()
    wk = nc.dram_tensor("wk", (_C, _HD), bf16, kind="ExternalInput").ap()
    wv = nc.dram_tensor("wv", (_C, _HD), bf16, kind="ExternalInput").ap()
    wo = nc.dram_tensor("wo", (_HD, _C), bf16, kind="ExternalInput").ap()
    cc = nc.dram_tensor("cc", (128, _T), bf16, kind="ExternalInput").ap()
    ssr = nc.dram_tensor("ssr", (128, _T), bf16, kind="ExternalInput").ap()
    tri = nc.dram_tensor("tri", (128, 128), bf16, kind="ExternalInput").ap()
    onesb = nc.dram_tensor("onesb", (128, 128), bf16, kind="ExternalInput").ap()
    out = nc.dram_tensor("out", (_T, _C), bf16, kind="ExternalOutput").ap()

    with tile.TileContext(nc) as tc, ExitStack() as ctx, \
            nc.allow_low_precision(reason="bf16 attention pipeline, f32 accum"):
        # ---------------- pools ----------------
        pc = ctx.enter_context(tc.tile_pool(name="pc", bufs=1))    # persistent
        pg = ctx.enter_context(tc.tile_pool(name="pg", bufs=2))    # scratch
        pa = ctx.enter_context(tc.tile_pool(name="pa", bufs=2))    # A tiles
        # PSUM: 2 + 2 + 3 + 1 = 8 banks
        ppj = ctx.enter_context(tc.tile_pool(name="ppj", bufs=2, space="PSUM"))
        pps = ctx.enter_context(tc.tile_pool(name="pps", bufs=2, space="PSUM"))
        ppo = ctx.enter_context(tc.tile_pool(name="ppo", bufs=3, space="PSUM"))
        ppd = ctx.enter_context(tc.tile_pool(name="ppd", bufs=1, space="PSUM"))

        # ---------------- persistent SBUF tiles + loads ----------------
        # single-DMA weight loads (3D APs) so the SP ring's fixed per-DMA
        # cost never gates the first projection groups.
        t_wkall = pc.tile([128, _NCB * _HD], bf16, tag="wkall")
        t_wqall = pc.tile([128, _NCB * _HD], bf16, tag="wqall")
        t_wvall = pc.tile([128, _NCB * _HD], bf16, tag="wvall")
        t_woall = pc.tile([128, _HPG * _C], bf16, tag="woall")
        t_wk = [t_wkall[:, c * _HD:(c + 1) * _HD] for c in range(_NCB)]
        t_wq = [t_wqall[:, c * _HD:(c + 1) * _HD] for c in range(_NCB)]
        t_wv = [t_wvall[:, c * _HD:(c + 1) * _HD] for c in range(_NCB)]
        t_wo = [t_woall[:, c * _C:(c + 1) * _C] for c in range(_HPG)]

        def _wload(tile_, dram, nchunks, width):
            nc.sync.dma_start(
                tile_[:].rearrange("p (c f) -> p c f", c=nchunks),
                dram.rearrange("(c p) f -> p c f", p=128))

        t_xt = [pc.tile([128, _T], bf16, tag=f"xt{c}", name=f"xt{c}")
                for c in range(_NCB)]
        t_cc = pc.tile([128, _T], bf16, tag="cc")
        t_ssr = pc.tile([128, _T], bf16, tag="ssr")
        t_tri = pc.tile([128, 128], bf16, tag="tri")
        t_ones = pc.tile([128, 128], bf16, tag="ones")

        def _xt_load(i):
            isl = slice(i * _TW, (i + 1) * _TW)
            for c in range(_NCB):
                nc.sync.dma_start(t_xt[c][:, isl], xT[c * 128:(c + 1) * 128, isl])

        _wload(t_wkall, wk, _NCB, _HD)
        _xt_load(0)
        _wload(t_wqall, wq, _NCB, _HD)
        nc.sync.dma_start(t_cc[:], cc[:])
        nc.sync.dma_start(t_ssr[:], ssr[:])
        nc.sync.dma_start(t_ones[:], onesb[:])
        _xt_load(1)
        _wload(t_wvall, wv, _NCB, _HD)
        _xt_load(2)
        nc.sync.dma_start(t_tri[:], tri[:])
        _wload(t_woall, wo, _HPG, _C)
        _xt_load(3)

        t_onescol = t_ones[:, 0:1]
        t_eps = pc.tile([128, 1], f32, tag="eps")
        nc.gpsimd.memset(t_eps[:], _EPS)
        # PE warmup during initial DMA wait: ramps the pstate clock and
        # fills otherwise-idle cycles; lands in the den bank (idle till qt0)
        t_warm = pc.tile([128, _TW], bf16, tag="warm")
        nc.gpsimd.memset(t_warm[:], 0.0)
        p_warm = ppd.tile([128, _TW], f32, tag="pd", name="warm")
        for _w in range(5):
            nc.tensor.matmul(p_warm[:], t_warm[:, 0:128], t_warm[:],
                             start=True, stop=True)

        # persistent per-head tensors
        t_kn = [pc.tile([128, _T], bf16, tag=f"kn{h}", name=f"kn{h}")
                for h in range(_HPG)]
        t_q = [pc.tile([128, _T], bf16, tag=f"q{h}", name=f"q{h}")
               for h in range(_HPG)]
        t_v = [pc.tile([128, _HD], bf16, tag=f"v{tb}", name=f"v{tb}")
               for tb in range(_NKC)]
        t_rk = [pc.tile([128, _NKC], f32, tag=f"rk{h}", name=f"rk{h}")
                for h in range(_HPG)]

        # ================ projections + rope + norms ================
        def proj_group(ws, h, isl, tag, halves=False):
            hsl = slice(h * 128, (h + 1) * 128)
            p = ppj.tile([128, _TW], f32, tag="pj", name=f"p{tag}")
            if halves:
                for colsl in (slice(0, 256), slice(256, 512)):
                    for c in range(_NCB):
                        nc.tensor.matmul(p[:, colsl], ws[c][:, hsl],
                                         t_xt[c][:, isl][:, colsl],
                                         start=(c == 0), stop=(c == _NCB - 1))
            else:
                for c in range(_NCB):
                    nc.tensor.matmul(p[:], ws[c][:, hsl], t_xt[c][:, isl],
                                     start=(c == 0), stop=(c == _NCB - 1))
            return p

        def rope_sq(dst_slice, p_raw, i, add_on_pool, evac_on_act):
            """dst = raw*cc + swap(raw)*ss (bf16); returns bf16 squares tile."""
            isl = slice(i * _TW, (i + 1) * _TW)
            t_raw = pg.tile([128, _TW], bf16, tag="raw", name="raw", bufs=8)
            if evac_on_act:
                nc.scalar.copy(t_raw[:], p_raw[:])
            else:
                nc.vector.tensor_copy(t_raw[:], p_raw[:])
            t_swm = pg.tile([128, _TW], bf16, tag="swm", name="swm", bufs=8)
            nc.vector.tensor_mul(t_swm[0:64, :], t_raw[64:128, :], t_ssr[64:128, isl])
            nc.vector.tensor_mul(t_swm[64:128, :], t_raw[0:64, :], t_ssr[0:64, isl])
            nc.vector.tensor_mul(dst_slice, t_raw[:], t_cc[:, isl])
            if add_on_pool:
                nc.gpsimd.tensor_add(dst_slice, dst_slice, t_swm[:])
            else:
                nc.vector.tensor_add(dst_slice, dst_slice, t_swm[:])
            t_sq = pg.tile([128, _TW], bf16, tag="sq", name="sq", bufs=8)
            if add_on_pool:
                nc.scalar.square(t_sq[:], dst_slice)
            else:
                nc.vector.tensor_mul(t_sq[:], dst_slice, dst_slice)
            return t_sq

        def k_chain_a(i, h):
            isl = slice(i * _TW, (i + 1) * _TW)
            p_k = proj_group(t_wk, h, isl, f"k{i}{h}")
            return rope_sq(t_kn[h][:, isl], p_k, i, add_on_pool=True, evac_on_act=True)

        def k_chain_b(i, h, t_sq):
            # ms_k columns (128,4) then rk = exp(-0.5*ln(ms_k))
            p_cols = pps.tile([128, 4], f32, tag="ps", name=f"cols{i}{h}",
                              padded_shape=[128, _TW])
            for j in range(4):
                nc.tensor.matmul(p_cols[:, j:j + 1], t_sq[:, j * 128:(j + 1) * 128],
                                 t_onescol, start=True, stop=True)
            t_lnk = pg.tile([128, 4], f32, tag="lnk", name="lnk", bufs=4)
            nc.scalar.activation(t_lnk[:], p_cols[:], Act.Ln,
                                 bias=t_eps[:], scale=1.0)
            nc.scalar.activation(t_rk[h][:, 4 * i:4 * i + 4], t_lnk[:], Act.Exp,
                                 bias=0.0, scale=-0.5)

        def q_chain(i, h):
            isl = slice(i * _TW, (i + 1) * _TW)
            p_q = proj_group(t_wq, h, isl, f"q{i}{h}")
            t_sq = rope_sq(t_q[h][:, isl], p_q, i, add_on_pool=False, evac_on_act=False)
            t_ms = pg.tile([128, _TW], bf16, tag="ms", name="ms", bufs=4)
            nc.gpsimd.partition_all_reduce(t_ms[:], t_sq[:], channels=128,
                                           reduce_op=bass_isa.ReduceOp.add)
            t_ln = pg.tile([128, _TW], bf16, tag="qln", name="qln", bufs=4)
            nc.scalar.activation(t_ln[:], t_ms[:], Act.Ln,
                                 bias=t_eps[:], scale=1.0 / 128.0)
            t_r = pg.tile([128, _TW], bf16, tag="qr", name="qr", bufs=4)
            nc.scalar.activation(t_r[:], t_ln[:], Act.Exp, bias=0.0, scale=-0.5)
            nc.vector.tensor_mul(t_q[h][:, isl], t_q[h][:, isl], t_r[:])

        def v_group(tb):
            bsl = slice(tb * 128, (tb + 1) * 128)
            p_v = ppj.tile([128, _HD], f32, tag="pj", name=f"pv{tb}",
                           padded_shape=[128, _TW])
            for c in range(_NCB):
                nc.tensor.matmul(p_v[:], t_xt[c][:, bsl], t_wv[c][:],
                                 start=(c == 0), stop=(c == _NCB - 1))
            if tb % 2 == 0:
                nc.vector.tensor_copy(t_v[tb][:], p_v[:])
            else:
                nc.scalar.copy(t_v[tb][:], p_v[:])

        # ================ attention + c_proj ================
        def attention(qt):
            nchunk = 4 * qt + 4
            LOOKAHEAD = 3
            qsl = slice(qt * _TW, (qt + 1) * _TW)
            p_den = ppd.tile([128, _TW], f32, tag="pd", name=f"pd{qt}")
            p_os = [ppo.tile([128, _TW], f32, tag="po", name=f"po{qt}{h}")
                    for h in range(_HPG)]
            a_tiles = {}

            def emit_s(kc, h):
                roff = 0 if kc < 4 * qt else (kc - 4 * qt) * 128
                nsl = slice(roff, _TW)
                ksl = slice(kc * 128, (kc + 1) * 128)
                p_s = pps.tile([128, _TW], f32, tag="ps", name=f"s{kc}{h}")
                nc.tensor.matmul(p_s[:, nsl], t_kn[h][:, ksl],
                                 t_q[h][:, qsl][:, nsl], start=True, stop=True)
                t_a = pa.tile([128, _TW], bf16, tag="a", name="a", bufs=32)
                nc.scalar.activation(t_a[:, nsl], p_s[:, nsl], Act.Exp,
                                     bias=0.0, scale=t_rk[h][:, kc:kc + 1])
                if kc >= 4 * qt:
                    dsl = slice(roff, roff + 128)
                    nc.vector.tensor_mul(t_a[:, dsl], t_a[:, dsl], t_tri[:])
                a_tiles[(kc, h)] = t_a

            def emit_acc(kc, h):
                roff = 0 if kc < 4 * qt else (kc - 4 * qt) * 128
                nsl = slice(roff, _TW)
                hsl = slice(h * 128, (h + 1) * 128)
                t_a = a_tiles.pop((kc, h))
                nc.tensor.matmul(p_den[32 * h:32 * h + 1, nsl], t_onescol,
                                 t_a[:, nsl],
                                 start=(kc == 0), stop=(kc == nchunk - 1))
                nc.tensor.matmul(p_os[h][:, nsl], t_v[kc][:, hsl], t_a[:, nsl],
                                 start=(kc == 0), stop=(kc == nchunk - 1))

            t_zs = [None] * _HPG

            def emit_z(h):
                t_rd = pg.tile([1, _TW], f32, tag="rd", name="rd", bufs=4)
                nc.vector.reciprocal(t_rd[:], p_den[32 * h:32 * h + 1, :])
                t_rdb = pg.tile([128, _TW], f32, tag="rdb", name="rdb", bufs=4)
                nc.gpsimd.partition_broadcast(t_rdb[:], t_rd[:])
                t_z = pg.tile([128, _TW], bf16, tag=f"z{h}", name=f"z{h}", bufs=2)
                nc.vector.tensor_mul(t_z[:], p_os[h][:], t_rdb[:])
                t_zs[h] = t_z

            # heads skewed by SKEW chunks so they finish staggered: each
            # head's z-chain (recip->bcast->mul, ~2.5us) runs while later
            # heads still stream chunks, freeing p_o banks incrementally.
            SKEW = 2 if qt < 2 else 4
            for v in range(nchunk + 2 * SKEW + LOOKAHEAD + 1):
                for h in range(_HPG):
                    kc_s = v - SKEW * h
                    if 0 <= kc_s < nchunk:
                        emit_s(kc_s, h)
                    kc_a = v - SKEW * h - LOOKAHEAD
                    if 0 <= kc_a < nchunk:
                        emit_acc(kc_a, h)
                        if kc_a == nchunk - 1:
                            emit_z(h)
            return t_zs

        def c_proj(qt, t_zs):
            for tb in range(4):
                bsl = slice(tb * 128, (tb + 1) * 128)
                r0 = qt * _TW + tb * 128
                t_ob = pg.tile([128, _C], bf16, tag="ob", name="ob", bufs=3)
                for nh in range(2):
                    osl = slice(nh * 384, (nh + 1) * 384)
                    p_c = ppj.tile([128, 384], f32, tag="pj", name=f"pc{qt}{tb}{nh}",
                                   padded_shape=[128, _TW])
                    for c in range(_HPG):
                        nc.tensor.matmul(p_c[:], t_zs[c][:, bsl], t_wo[c][:, osl],
                                         start=(c == 0), stop=(c == _HPG - 1))
                    if qt == _NT - 1 and nh == 1:
                        nc.scalar.copy(t_ob[:, osl], p_c[:])
                    else:
                        nc.vector.tensor_copy(t_ob[:, osl], p_c[:])
                eng = nc.sync if tb % 2 == 0 else nc.scalar
                eng.dma_start(out[r0:r0 + 128, :], t_ob[:])

        # ---------------- emission ----------------
        for i in range(_NT):
            k_sqs = [k_chain_a(i, h) for h in range(_HPG)]
            for h in range(_HPG):
                q_chain(i, h)
            for tb in range(4 * i, 4 * i + 4):
                v_group(tb)
            for h in range(_HPG):
                k_chain_b(i, h, k_sqs[h])

        for qt in range(_NT):
            t_zs = attention(qt)
            c_proj(qt, t_zs)

    nc.compile()
    return nc


def _get_nc():
    if "nc" not in _cached:
        _cached["nc"] = _build_nc()
    return _cached["nc"]


def make_in_maps(x, cos, sin, Wq, Wk, Wv, Wo):
    import ml_dtypes
    bf = ml_dtypes.bfloat16
    cosT = np.ascontiguousarray(cos.reshape(_T, _D // 2).T)  # (64, T)
    sinT = np.ascontiguousarray(sin.reshape(_T, _D // 2).T)
    ccm = np.concatenate([cosT, cosT], axis=0).astype(bf)     # (128, T)
    ssm = np.concatenate([sinT, -sinT], axis=0).astype(bf)
    ssrm = np.concatenate([-sinT, sinT], axis=0).astype(bf)
    trim = (np.arange(128)[None, :] >= np.arange(128)[:, None]).astype(bf)
    ones128 = np.ones((128, 128), dtype=bf)
    in_maps = []
    for core in range(8):
        b, g = divmod(core, 2)
        gsl = slice(g * _HD, (g + 1) * _HD)
        in_maps.append({
            "xT": np.ascontiguousarray(x[b].T).astype(bf),
            "wq": np.ascontiguousarray(Wq[gsl, :].T).astype(bf),
            "wk": np.ascontiguousarray(Wk[gsl, :].T).astype(bf),
            "wv": np.ascontiguousarray(Wv[gsl, :].T).astype(bf),
            "wo": np.ascontiguousarray(Wo[:, gsl].T).astype(bf),
            "cc": ccm, "ssr": ssrm, "tri": trim, "onesb": ones128,
        })
    return in_maps


def kernel(x, cos, sin, Wq, Wk, Wv, Wo):
    from concourse.bass_utils import run_bass_kernel_spmd

    x = np.asarray(x, dtype=np.float32)
    cos = np.asarray(cos, dtype=np.float32)
    sin = np.asarray(sin, dtype=np.float32)
    Wq = np.asarray(Wq, dtype=np.float32)
    Wk = np.asarray(Wk, dtype=np.float32)
    Wv = np.asarray(Wv, dtype=np.float32)
    Wo = np.asarray(Wo, dtype=np.float32)

    nc = _get_nc()
    in_maps = make_in_maps(x, cos, sin, Wq, Wk, Wv, Wo)
    res = run_bass_kernel_spmd(nc, in_maps, core_ids=list(range(8)))
    outs = [np.asarray(r_["out"], dtype=np.float32) for r_ in res.results]
    return np.stack([outs[2 * b] + outs[2 * b + 1] for b in range(_B)], axis=0)


# revision 8
# speedup vs baseline: 1.9734x; 1.0078x over previous
"""Trainium2 Bass kernel for CausalSelfAttention (B=4, T=2048, C=768, H=6, D=128)
with RoPE + QK-RMSNorm.  v3: one act-table set, soft-pipelined phases.

Sharding: 8 cores = batch(4) x head-group(2, 3 heads each).

Key points:
  - Q/K/A/V tiles bf16 (DVE 2x, same PE rate); projections/c_proj f32r.
  - RoPE half-swap via partition-offset DVE muls (no PE perm matmul).
  - K-side RMSNorm never scales K: rk = 1/sqrt(ms_k) columns (with 1/sqrt(D)
    folded in) feed exp's per-partition scale.  ms_k columns computed directly
    via matmul(lhsT=sq_chunk, rhs=ones_col) at psum partitions.
  - Q-side RMSNorm via gpsimd partition_all_reduce.
  - rsqrt = exp(-0.5*ln(x)) on Act; every activation func lives in act-table
    set 'natural_log_exp_and_others', pinned via a filtered table view, so
    the scheduler can interleave norm chains with attention exps freely with
    zero table reloads.
  - Softmax den rows for 3 heads pack one PSUM bank at partitions 0/32/64.
"""

import numpy as np

_B, _T, _C, _H, _D = 4, 2048, 768, 6, 128
_HPG = 3            # heads per group (per core)
_HD = _HPG * _D     # 384
_NT = 4             # T tiles of 512
_TW = 512
_NKC = _T // 128    # 16 k-chunks
_NCB = _C // 128    # 6 contraction chunks
_EPS = 1e-15

_cached = {}


def _patch_act_tables():
    """Pin every activation func we use to the 'natural_log_exp_and_others'
    table: present a filtered view to Bacc's table-load pass in which a func
    appears in a non-6 set only if set 6 cannot serve it.  Runtime behavior is
    unchanged (the real set 6 does contain ln/exp/copy/square/identity); this
    only steers load placement so ln/exp alternation never reloads."""
    import concourse.bacc as bacc_mod
    import concourse.hw_specs as hw_mod
    if getattr(bacc_mod, "_act_tables_patched", False):
        return
    orig = hw_mod.get_activation_tables

    def patched(arch):
        tables = orig(arch)
        items = list(tables.items())
        target = None
        for name, funcs in items:
            if name == "natural_log_exp_and_others":
                target = funcs
        if target is None:
            return tables
        out = {}
        for name, funcs in items:
            if name == "natural_log_exp_and_others":
                out[name] = funcs
            else:
                out[name] = {f for f in funcs if f not in target}
        return out

    bacc_mod.get_activation_tables = patched
    bacc_mod._act_tables_patched = True


def _build_nc():
    from contextlib import ExitStack
    from concourse import bacc, tile, mybir, bass_isa

    _patch_act_tables()

    f32 = mybir.dt.float32
    f32r = mybir.dt.float32r
    bf16 = mybir.dt.bfloat16
    Act = mybir.ActivationFunctionType

    nc = bacc.Bacc("TRN2", target_bir_lowering=False, debug=False)

    xT = nc.dram_tensor("xT", (_C, _T), bf16, kind="ExternalInput").ap()
    wq = nc.dram_tensor("wq", (_C, _HD), bf16, kind="ExternalInput").ap()
    wk = nc.dram_tensor("wk", (_C, _HD), bf16, kind="ExternalInput").ap()
    wv = nc.dram_tensor("wv", (_C, _HD), bf16, kind="ExternalInput").ap()
    wo = nc.dram_tensor("wo", (_HD, _C), bf16, kind="ExternalInput").ap()
    cc = nc.dram_tensor("cc", (128, _T), bf16, kind="ExternalInput").ap()
    ssr = nc.dram_tensor("ssr", (128, _T), bf16, kind="ExternalInput").ap()
    tri = nc.dram_tensor("tri", (128, 128), bf16, kind="ExternalInput").ap()
    onesb = nc.dram_tensor("onesb", (128, 128), bf16, kind="ExternalInput").ap()
    out = nc.dram_tensor("out", (_T, _C), bf16, kind="ExternalOutput").ap()

    with tile.TileContext(nc) as tc, ExitStack() as ctx, \
            nc.allow_low_precision(reason="bf16 attention pipeline, f32 accum"):
        # ---------------- pools ----------------
        pc = ctx.enter_context(tc.tile_pool(name="pc", bufs=1))    # persistent
        pg = ctx.enter_context(tc.tile_pool(name="pg", bufs=2))    # scratch
        pa = ctx.enter_context(tc.tile_pool(name="pa", bufs=2))    # A tiles
        # PSUM: 2 + 2 + 3 + 1 = 8 banks
        ppj = ctx.enter_context(tc.tile_pool(name="ppj", bufs=2, space="PSUM"))
        pps = ctx.enter_context(tc.tile_pool(name="pps", bufs=2, space="PSUM"))
        ppo = ctx.enter_context(tc.tile_pool(name="ppo", bufs=3, space="PSUM"))
        ppd = ctx.enter_context(tc.tile_pool(name="ppd", bufs=1, space="PSUM"))

        # ---------------- persistent SBUF tiles + loads ----------------
        # single-DMA weight loads (3D APs) so the SP ring's fixed per-DMA
        # cost never gates the first projection groups.
        t_wkall = pc.tile([128, _NCB * _HD], bf16, tag="wkall")
        t_wqall = pc.tile([128, _NCB * _HD], bf16, tag="wqall")
        t_wvall = pc.tile([128, _NCB * _HD], bf16, tag="wvall")
        t_woall = pc.tile([128, _HPG * _C], bf16, tag="woall")
        t_wk = [t_wkall[:, c * _HD:(c + 1) * _HD] for c in range(_NCB)]
        t_wq = [t_wqall[:, c * _HD:(c + 1) * _HD] for c in range(_NCB)]
        t_wv = [t_wvall[:, c * _HD:(c + 1) * _HD] for c in range(_NCB)]
        t_wo = [t_woall[:, c * _C:(c + 1) * _C] for c in range(_HPG)]

        def _wload(tile_, dram, nchunks, width):
            nc.sync.dma_start(
                tile_[:].rearrange("p (c f) -> p c f", c=nchunks),
                dram.rearrange("(c p) f -> p c f", p=128))

        t_xt = [pc.tile([128, _T], bf16, tag=f"xt{c}", name=f"xt{c}")
                for c in range(_NCB)]
        t_cc = pc.tile([128, _T], bf16, tag="cc")
        t_ssr = pc.tile([128, _T], bf16, tag="ssr")
        t_tri = pc.tile([128, 128], bf16, tag="tri")
        t_ones = pc.tile([128, 128], bf16, tag="ones")

        def _xt_load(i):
            isl = slice(i * _TW, (i + 1) * _TW)
            for c in range(_NCB):
                nc.sync.dma_start(t_xt[c][:, isl], xT[c * 128:(c + 1) * 128, isl])

        def _cs_load(i):
            isl = slice(i * _TW, (i + 1) * _TW)
            nc.sync.dma_start(t_cc[:, isl], cc[:, isl])
            nc.sync.dma_start(t_ssr[:, isl], ssr[:, isl])

        _wload(t_wkall, wk, _NCB, _HD)
        _xt_load(0)
        _wload(t_wqall, wq, _NCB, _HD)
        _cs_load(0)
        nc.sync.dma_start(t_ones[:], onesb[:])
        _xt_load(1)
        _wload(t_wvall, wv, _NCB, _HD)
        _cs_load(1)
        _xt_load(2)
        nc.sync.dma_start(t_tri[:], tri[:])
        _cs_load(2)
        _wload(t_woall, wo, _HPG, _C)
        _xt_load(3)
        _cs_load(3)

        t_onescol = t_ones[:, 0:1]
        t_eps = pc.tile([128, 1], f32, tag="eps")
        nc.gpsimd.memset(t_eps[:], _EPS)
        # PE warmup during initial DMA wait: ramps the pstate clock and
        # fills otherwise-idle cycles; lands in the den bank (idle till qt0)
        t_warm = pc.tile([128, _TW], bf16, tag="warm")
        nc.gpsimd.memset(t_warm[:], 0.0)
        p_warm = ppd.tile([128, _TW], f32, tag="pd", name="warm")
        for _w in range(5):
            nc.tensor.matmul(p_warm[:], t_warm[:, 0:128], t_warm[:],
                             start=True, stop=True)

        # persistent per-head tensors
        t_kn = [pc.tile([128, _T], bf16, tag=f"kn{h}", name=f"kn{h}")
                for h in range(_HPG)]
        t_q = [pc.tile([128, _T], bf16, tag=f"q{h}", name=f"q{h}")
               for h in range(_HPG)]
        t_v = [pc.tile([128, _HD], bf16, tag=f"v{tb}", name=f"v{tb}")
               for tb in range(_NKC)]
        t_rk = [pc.tile([128, _NKC], f32, tag=f"rk{h}", name=f"rk{h}")
                for h in range(_HPG)]

        # ================ projections + rope + norms ================
        def proj_group(ws, h, isl, tag, halves=False):
            hsl = slice(h * 128, (h + 1) * 128)
            p = ppj.tile([128, _TW], f32, tag="pj", name=f"p{tag}")
            if halves:
                for colsl in (slice(0, 256), slice(256, 512)):
                    for c in range(_NCB):
                        nc.tensor.matmul(p[:, colsl], ws[c][:, hsl],
                                         t_xt[c][:, isl][:, colsl],
                                         start=(c == 0), stop=(c == _NCB - 1))
            else:
                for c in range(_NCB):
                    nc.tensor.matmul(p[:], ws[c][:, hsl], t_xt[c][:, isl],
                                     start=(c == 0), stop=(c == _NCB - 1))
            return p

        def rope_sq(dst_slice, p_raw, i, add_on_pool, evac_on_act):
            """dst = raw*cc + swap(raw)*ss (bf16); returns bf16 squares tile."""
            isl = slice(i * _TW, (i + 1) * _TW)
            t_raw = pg.tile([128, _TW], bf16, tag="raw", name="raw", bufs=8)
            if evac_on_act:
                nc.scalar.copy(t_raw[:], p_raw[:])
            else:
                nc.vector.tensor_copy(t_raw[:], p_raw[:])
            t_swm = pg.tile([128, _TW], bf16, tag="swm", name="swm", bufs=8)
            nc.vector.tensor_mul(t_swm[0:64, :], t_raw[64:128, :], t_ssr[64:128, isl])
            nc.vector.tensor_mul(t_swm[64:128, :], t_raw[0:64, :], t_ssr[0:64, isl])
            nc.vector.tensor_mul(dst_slice, t_raw[:], t_cc[:, isl])
            if add_on_pool:
                nc.gpsimd.tensor_add(dst_slice, dst_slice, t_swm[:])
            else:
                nc.vector.tensor_add(dst_slice, dst_slice, t_swm[:])
            t_sq = pg.tile([128, _TW], bf16, tag="sq", name="sq", bufs=8)
            if add_on_pool:
                nc.scalar.square(t_sq[:], dst_slice)
            else:
                nc.vector.tensor_mul(t_sq[:], dst_slice, dst_slice)
            return t_sq

        def k_chain_a(i, h):
            isl = slice(i * _TW, (i + 1) * _TW)
            p_k = proj_group(t_wk, h, isl, f"k{i}{h}")
            return rope_sq(t_kn[h][:, isl], p_k, i, add_on_pool=True, evac_on_act=True)

        def k_chain_b(i, h, t_sq):
            # ms_k columns (128,4) then rk = exp(-0.5*ln(ms_k))
            p_cols = pps.tile([128, 4], f32, tag="ps", name=f"cols{i}{h}",
                              padded_shape=[128, _TW])
            for j in range(4):
                nc.tensor.matmul(p_cols[:, j:j + 1], t_sq[:, j * 128:(j + 1) * 128],
                                 t_onescol, start=True, stop=True)
            t_lnk = pg.tile([128, 4], f32, tag="lnk", name="lnk", bufs=4)
            nc.scalar.activation(t_lnk[:], p_cols[:], Act.Ln,
                                 bias=t_eps[:], scale=1.0)
            nc.scalar.activation(t_rk[h][:, 4 * i:4 * i + 4], t_lnk[:], Act.Exp,
                                 bias=0.0, scale=-0.5)

        def q_chain(i, h):
            isl = slice(i * _TW, (i + 1) * _TW)
            p_q = proj_group(t_wq, h, isl, f"q{i}{h}")
            t_sq = rope_sq(t_q[h][:, isl], p_q, i, add_on_pool=False, evac_on_act=False)
            t_ms = pg.tile([128, _TW], bf16, tag="ms", name="ms", bufs=4)
            nc.gpsimd.partition_all_reduce(t_ms[:], t_sq[:], channels=128,
                                           reduce_op=bass_isa.ReduceOp.add)
            t_ln = pg.tile([128, _TW], bf16, tag="qln", name="qln", bufs=4)
            nc.scalar.activation(t_ln[:], t_ms[:], Act.Ln,
                                 bias=t_eps[:], scale=1.0 / 128.0)
            t_r = pg.tile([128, _TW], bf16, tag="qr", name="qr", bufs=4)
            nc.scalar.activation(t_r[:], t_ln[:], Act.Exp, bias=0.0, scale=-0.5)
            nc.vector.tensor_mul(t_q[h][:, isl], t_q[h][:, isl], t_r[:])

        def v_group(tb):
            bsl = slice(tb * 128, (tb + 1) * 128)
            p_v = ppj.tile([128, _HD], f32, tag="pj", name=f"pv{tb}",
                           padded_shape=[128, _TW])
            for c in range(_NCB):
                nc.tensor.matmul(p_v[:], t_xt[c][:, bsl], t_wv[c][:],
                                 start=(c == 0), stop=(c == _NCB - 1))
            if tb % 2 == 0:
                nc.vector.tensor_copy(t_v[tb][:], p_v[:])
            else:
                nc.scalar.copy(t_v[tb][:], p_v[:])

        # ================ attention + c_proj ================
        def attention(qt):
            nchunk = 4 * qt + 4
            LOOKAHEAD = 3
            qsl = slice(qt * _TW, (qt + 1) * _TW)
            p_den = ppd.tile([128, _TW], f32, tag="pd", name=f"pd{qt}")
            p_os = [ppo.tile([128, _TW], f32, tag="po", name=f"po{qt}{h}")
                    for h in range(_HPG)]
            a_tiles = {}

            def emit_s(kc, h):
                roff = 0 if kc < 4 * qt else (kc - 4 * qt) * 128
                nsl = slice(roff, _TW)
                ksl = slice(kc * 128, (kc + 1) * 128)
                p_s = pps.tile([128, _TW], f32, tag="ps", name=f"s{kc}{h}")
                nc.tensor.matmul(p_s[:, nsl], t_kn[h][:, ksl],
                                 t_q[h][:, qsl][:, nsl], start=True, stop=True)
                t_a = pa.tile([128, _TW], bf16, tag="a", name="a", bufs=32)
                nc.scalar.activation(t_a[:, nsl], p_s[:, nsl], Act.Exp,
                                     bias=0.0, scale=t_rk[h][:, kc:kc + 1])
                if kc >= 4 * qt:
                    dsl = slice(roff, roff + 128)
                    nc.vector.tensor_mul(t_a[:, dsl], t_a[:, dsl], t_tri[:])
                a_tiles[(kc, h)] = t_a

            def emit_acc(kc, h):
                roff = 0 if kc < 4 * qt else (kc - 4 * qt) * 128
                nsl = slice(roff, _TW)
                hsl = slice(h * 128, (h + 1) * 128)
                t_a = a_tiles.pop((kc, h))
                nc.tensor.matmul(p_den[32 * h:32 * h + 1, nsl], t_onescol,
                                 t_a[:, nsl],
                                 start=(kc == 0), stop=(kc == nchunk - 1))
                nc.tensor.matmul(p_os[h][:, nsl], t_v[kc][:, hsl], t_a[:, nsl],
                                 start=(kc == 0), stop=(kc == nchunk - 1))

            t_zs = [None] * _HPG

            def emit_z(h):
                t_rd = pg.tile([1, _TW], f32, tag="rd", name="rd", bufs=4)
                nc.vector.reciprocal(t_rd[:], p_den[32 * h:32 * h + 1, :])
                t_rdb = pg.tile([128, _TW], f32, tag="rdb", name="rdb", bufs=4)
                nc.gpsimd.partition_broadcast(t_rdb[:], t_rd[:])
                t_z = pg.tile([128, _TW], bf16, tag=f"z{h}", name=f"z{h}", bufs=2)
                nc.vector.tensor_mul(t_z[:], p_os[h][:], t_rdb[:])
                t_zs[h] = t_z

            # heads skewed by SKEW chunks so they finish staggered: each
            # head's z-chain (recip->bcast->mul, ~2.5us) runs while later
            # heads still stream chunks, freeing p_o banks incrementally.
            SKEW = [2, 2, 3, 4][qt]
            for v in range(nchunk + 2 * SKEW + LOOKAHEAD + 1):
                for h in range(_HPG):
                    kc_s = v - SKEW * h
                    if 0 <= kc_s < nchunk:
                        emit_s(kc_s, h)
                    kc_a = v - SKEW * h - LOOKAHEAD
                    if 0 <= kc_a < nchunk:
                        emit_acc(kc_a, h)
                        if kc_a == nchunk - 1:
                            emit_z(h)
            return t_zs

        def c_proj(qt, t_zs):
            for tb in range(4):
                bsl = slice(tb * 128, (tb + 1) * 128)
                r0 = qt * _TW + tb * 128
                t_ob = pg.tile([128, _C], bf16, tag="ob", name="ob", bufs=3)
                for nh in range(2):
                    osl = slice(nh * 384, (nh + 1) * 384)
                    p_c = ppj.tile([128, 384], f32, tag="pj", name=f"pc{qt}{tb}{nh}",
                                   padded_shape=[128, _TW])
                    for c in range(_HPG):
                        nc.tensor.matmul(p_c[:], t_zs[c][:, bsl], t_wo[c][:, osl],
                                         start=(c == 0), stop=(c == _HPG - 1))
                    if qt == _NT - 1 and nh == 1:
                        nc.scalar.copy(t_ob[:, osl], p_c[:])
                    else:
                        nc.vector.tensor_copy(t_ob[:, osl], p_c[:])
                eng = nc.sync if tb % 2 == 0 else nc.scalar
                eng.dma_start(out[r0:r0 + 128, :], t_ob[:])

        # ---------------- emission ----------------
        for i in range(_NT):
            k_sqs = [k_chain_a(i, h) for h in range(_HPG)]
            for h in range(_HPG):
                q_chain(i, h)
            for tb in range(4 * i, 4 * i + 4):
                v_group(tb)
            for h in range(_HPG):
                k_chain_b(i, h, k_sqs[h])

        for qt in range(_NT):
            t_zs = attention(qt)
            c_proj(qt, t_zs)

    nc.compile()
    return nc


def _get_nc():
    if "nc" not in _cached:
        _cached["nc"] = _build_nc()
    return _cached["nc"]


def make_in_maps(x, cos, sin, Wq, Wk, Wv, Wo):
    import ml_dtypes
    bf = ml_dtypes.bfloat16
    cosT = np.ascontiguousarray(cos.reshape(_T, _D // 2).T)  # (64, T)
    sinT = np.ascontiguousarray(sin.reshape(_T, _D // 2).T)
    ccm = np.concatenate([cosT, cosT], axis=0).astype(bf)     # (128, T)
    ssm = np.concatenate([sinT, -sinT], axis=0).astype(bf)
    ssrm = np.concatenate([-sinT, sinT], axis=0).astype(bf)
    trim = (np.arange(128)[None, :] >= np.arange(128)[:, None]).astype(bf)
    ones128 = np.ones((128, 128), dtype=bf)
    in_maps = []
    for core in range(8):
        b, g = divmod(core, 2)
        gsl = slice(g * _HD, (g + 1) * _HD)
        in_maps.append({
            "xT": np.ascontiguousarray(x[b].T).astype(bf),
            "wq": np.ascontiguousarray(Wq[gsl, :].T).astype(bf),
            "wk": np.ascontiguousarray(Wk[gsl, :].T).astype(bf),
            "wv": np.ascontiguousarray(Wv[gsl, :].T).astype(bf),
            "wo": np.ascontiguousarray(Wo[:, gsl].T).astype(bf),
            "cc": ccm, "ssr": ssrm, "tri": trim, "onesb": ones128,
        })
    return in_maps


def kernel(x, cos, sin, Wq, Wk, Wv, Wo):
    from concourse.bass_utils import run_bass_kernel_spmd

    x = np.asarray(x, dtype=np.float32)
    cos = np.asarray(cos, dtype=np.float32)
    sin = np.asarray(sin, dtype=np.float32)
    Wq = np.asarray(Wq, dtype=np.float32)
    Wk = np.asarray(Wk, dtype=np.float32)
    Wv = np.asarray(Wv, dtype=np.float32)
    Wo = np.asarray(Wo, dtype=np.float32)

    nc = _get_nc()
    in_maps = make_in_maps(x, cos, sin, Wq, Wk, Wv, Wo)
    res = run_bass_kernel_spmd(nc, in_maps, core_ids=list(range(8)))
    outs = [np.asarray(r_["out"], dtype=np.float32) for r_ in res.results]
    return np.stack([outs[2 * b] + outs[2 * b + 1] for b in range(_B)], axis=0)
